# revision 4
# baseline (speedup 1.0000x reference)
"""HGT graph update kernel for 8 Trainium2 NeuronCores.

Strategy:
  * Host folds the per-relation projections into node-level weights:
      kt_s = x @ (Wk @ blockdiag(Watt_s)) * prior_s/sqrt(C)   (per head col-block)
      mt_s = x @ (Wm @ blockdiag(Wmsg_s))
    so each edge only needs gathers:  score = <kt_s[src], q[dst]>_per-head,
    msg = mt_s[src].
  * Softmax without the max-subtraction pass (scores are O(1) here; the
    shifted/unshifted softmax are algebraically identical, fp32-safe).
  * All 2E edges are sorted by destination on the host; the 8 cores own
    contiguous 12500-node ranges, so each core completes its own segment
    softmax locally - the only collective is one AllGather of the node
    tables kt/mt (q stays core-local in SBUF).
  * Edge phase: per 128-edge block, one indirect DMA gathers [kt|mt]
    (1024B/edge) from the gathered table; q[dst] is reconstructed with a
    one-hot matmul from SBUF (no DMA); scatter-add into a PSUM window of
    128 consecutive dst nodes via a one-hot matmul.

  Host<->device transport (the axon tunnel, ~50MB/s) dominates, so:
  * x and the output travel as fp16 (tolerance is 2e-2; fp16 adds ~5e-4).
  * Each edge is packed into ONE int32: kmidx (18b) | rowlocal (8b).
    rowlocal is unpacked on device (shift + int->f32 copy); the one-hot
    selector transpose is done on TensorE instead of shipping a second
    pre-transposed copy of the indices.
  * Weights/biases ship as fp16, biases as single rows broadcast on
    device via a ones-outer-product matmul.
  * The jitted PJRT callable is built once and cached; the output-operand
    required by the bass_exec custom call is a persistent device-resident
    dummy (the kernel writes every output element), so no per-call zeros
    upload.
"""

import sys

if "/opt/trn_rl_repo" not in sys.path:
    sys.path.insert(0, "/opt/trn_rl_repo")
import numpy as np

N, D, H, C = 100000, 128, 8, 16
LN_EPS = 1e-3
NCORES = 8
P = 128
ROWSHIFT = 18                 # kmidx bits 0..17, rowlocal bits 18..25
KMASK = (1 << ROWSHIFT) - 1
DUMMY_ROW = 255               # never matches a 0..127 local row


def _host_prep(x, src0, dst0, src1, dst1, Wk, bk, Wm, bm, Wq, bq, Wa, ba,
               Watt0, Wmsg0, Watt1, Wmsg1, prior0, prior1, skip, gamma, beta):
    """Fold weights, sort edges by dst, build per-core packed index records."""
    f32 = np.float32
    x = np.asarray(x, f32)
    n = x.shape[0]
    npc = n // NCORES            # nodes per core
    nwin = (npc + P - 1) // P    # windows (128-node groups) per core

    def bd(w):  # [H,C,C] -> block-diagonal [D,D]
        out = np.zeros((H * C, H * C), f32)
        for h in range(H):
            out[h * C:(h + 1) * C, h * C:(h + 1) * C] = np.asarray(w[h], f32)
        return out

    scale = 1.0 / np.sqrt(f32(C))
    cs0 = np.repeat(np.asarray(prior0, f32) * scale, C)   # [D] col scale
    cs1 = np.repeat(np.asarray(prior1, f32) * scale, C)
    Wk, bk, Wm, bm = (np.asarray(a, f32) for a in (Wk, bk, Wm, bm))
    Wkt0 = (Wk @ bd(Watt0)) * cs0; bkt0 = (bk @ bd(Watt0)) * cs0
    Wkt1 = (Wk @ bd(Watt1)) * cs1; bkt1 = (bk @ bd(Watt1)) * cs1
    Wmt0 = Wm @ bd(Wmsg0); bmt0 = bm @ bd(Wmsg0)
    Wmt1 = Wm @ bd(Wmsg1); bmt1 = bm @ bd(Wmsg1)
    # T row layout per node: [kt0 | mt0 | kt1 | mt1]  -> viewed as [2n, 256]:
    # row 2s+b = [kt_b | mt_b] of node s.
    Wbig = np.concatenate([Wkt0, Wmt0, Wkt1, Wmt1], axis=1)        # [128, 512]
    bbig = np.concatenate([bkt0, bmt0, bkt1, bmt1])                # [512]

    # ---- edges: sort by dst ----
    e0 = len(np.asarray(src0)); e1 = len(np.asarray(src1))
    src = np.concatenate([np.asarray(src0), np.asarray(src1)]).astype(np.int64)
    dst = np.concatenate([np.asarray(dst0), np.asarray(dst1)]).astype(np.int64)
    eset = np.concatenate([np.zeros(e0, np.int64), np.ones(e1, np.int64)])
    order = np.argsort(dst, kind="stable")
    src, dst, eset = src[order], dst[order], eset[order]
    kmidx = (2 * src + eset).astype(np.int64)      # row into [2n, 256] table

    # per-core, per-window edge ranges: window w of core c covers nodes
    # [c*npc + w*P, next boundary); boundaries are strictly increasing so a
    # single searchsorted over all of them yields every window's edge range.
    win_start = np.asarray(
        [c * npc + w * P for c in range(NCORES) for w in range(nwin)], np.int64)
    starts = np.searchsorted(dst, np.concatenate([win_start, [n]]), side="left")
    cnts = np.diff(starts)                           # edges per window
    bpw = max(1, int((cnts.max() + P - 1) // P))

    # packed record: word = kmidx | rowlocal<<18, block-major within window
    rowloc = dst - np.repeat(win_start, cnts)
    word_all = (kmidx | (rowloc << ROWSHIFT)).astype(np.int32)
    wrec = np.full((NCORES * nwin, bpw * P), DUMMY_ROW << ROWSHIFT, np.int32)
    mask = np.arange(bpw * P)[None, :] < cnts[:, None]
    wrec[mask] = word_all
    # [nwin*P? ] -> per window [bpw, P] -> transpose to [P, bpw]
    wrec = wrec.reshape(NCORES, nwin, bpw, P).transpose(0, 1, 3, 2)
    wrec = np.ascontiguousarray(wrec)

    alpha = float(1.0 / (1.0 + np.exp(-np.float64(np.asarray(skip)))))
    f16 = np.float16
    wall = np.concatenate(
        [Wbig, np.asarray(Wq, f32), np.asarray(Wa, f32)], axis=1).astype(f16)
    crow = np.concatenate(
        [bbig, np.asarray(bq, f32), np.asarray(ba, f32) * f32(alpha),
         np.asarray(gamma, f32), np.asarray(beta, f32)])[None, :].astype(f16)
    consts = dict(wall=wall, crow=crow)
    xh = x.astype(f16)
    in_maps = []
    for c in range(NCORES):
        m = dict(consts)
        m["x_half"] = np.ascontiguousarray(xh[c * npc:(c + 1) * npc])
        m["wrec"] = wrec[c]
        in_maps.append(m)
    return in_maps, dict(n=n, npc=npc, nwin=nwin, bpw=bpw, alpha=alpha)


def _build(meta):
    """Build the Bass program (shared by all 8 cores)."""
    import concourse.bass as bass
    import concourse.mybir as mybir
    import concourse.tile as tile
    from concourse.masks import make_identity

    f32 = mybir.dt.float32
    f16 = mybir.dt.float16
    i32 = mybir.dt.int32
    AF = mybir.ActivationFunctionType
    OP = mybir.AluOpType
    n, npc, nwin, bpw = meta["n"], meta["npc"], meta["nwin"], meta["bpw"]
    alpha = meta["alpha"]

    import concourse.bacc as bacc
    nc = bacc.Bacc(trn_type="TRN2", num_devices=NCORES)

    x_half = nc.dram_tensor("x_half", [npc, D], f16, kind="ExternalInput")
    wrec = nc.dram_tensor("wrec", [nwin, P, bpw], i32, kind="ExternalInput")
    wall = nc.dram_tensor("wall", [D, 6 * D], f16, kind="ExternalInput")
    crow = nc.dram_tensor("crow", [1, 8 * D], f16, kind="ExternalInput")
    out = nc.dram_tensor("out", [npc, D], f16, kind="ExternalOutput")

    from contextlib import ExitStack
    with tile.TileContext(nc, num_cores=NCORES) as tc:
        with (
            tc.tile_pool(name="const", bufs=1) as cpool,
            tc.tile_pool(name="dram", bufs=1, space="DRAM") as dram,
        ):
            # ---- constants ----
            identity = cpool.tile([P, P], f32)
            make_identity(nc, identity[:])
            iota_free = cpool.tile([P, P], f32)
            nc.gpsimd.iota(iota_free[:], pattern=[[1, P]], channel_multiplier=0,
                           allow_small_or_imprecise_dtypes=True)
            ones_row = cpool.tile([1, P], f32)
            nc.vector.memset(ones_row[:], 1.0)
            zero_col = cpool.tile([P, 1], f32)
            nc.vector.memset(zero_col[:], 0.0)
            eps_col = cpool.tile([P, 1], f32)
            nc.vector.memset(eps_col[:], LN_EPS)
            nc.const_aps.aps[(f32, 0.0)] = zero_col[:]
            nc.const_aps.aps[(f32, LN_EPS)] = eps_col[:]

            wall_h = cpool.tile([D, 6 * D], f16)
            nc.sync.dma_start(wall_h[:], wall[:])
            wall_t = cpool.tile([D, 6 * D], f32)
            nc.scalar.copy(wall_t[:], wall_h[:])
            wbig_t = wall_t[:][:, 0:4 * D]
            wq_t = wall_t[:][:, 4 * D:5 * D]
            wa_t = wall_t[:][:, 5 * D:6 * D]

            crow_h = cpool.tile([1, 8 * D], f16)
            nc.sync.dma_start(crow_h[:], crow[:])
            crow_f = cpool.tile([1, 8 * D], f32)
            nc.scalar.copy(crow_f[:], crow_h[:])
            cbrd = cpool.tile([P, 8 * D], f32)
            with tc.tile_pool(name="bc_ps", bufs=2, space="PSUM") as bcps:
                for j in range(2):
                    ps = bcps.tile([P, 4 * D], f32, tag="bc")
                    nc.tensor.matmul(ps[:], lhsT=ones_row[:],
                                     rhs=crow_f[:][:, j * 4 * D:(j + 1) * 4 * D],
                                     start=True, stop=True)
                    nc.scalar.copy(cbrd[:, j * 4 * D:(j + 1) * 4 * D], ps[:])
            bb_t = cbrd[:][:, 0:4 * D]
            bq_t = cbrd[:][:, 4 * D:5 * D]
            baa_t = cbrd[:][:, 5 * D:6 * D]
            gam_t = cbrd[:][:, 6 * D:7 * D]
            bet_t = cbrd[:][:, 7 * D:8 * D]

            # persistent SBUF state
            q_sbuf = cpool.tile([P, nwin * D], f32)
            nc.gpsimd.memset(q_sbuf[:], 0)
            pooled = cpool.tile([P, nwin * 136], f32)

            T_local = dram.tile([npc, 4 * D], f32)
            T_full = dram.tile([2 * n, 2 * D], f32)

            # ================= Phase A: projections =================
            stkA = ExitStack()
            apool = stkA.enter_context(tc.tile_pool(name="a_sb", bufs=3))
            apsum = stkA.enter_context(tc.tile_pool(name="a_ps", bufs=2, space="PSUM"))
            for t in range(nwin):
                nt = min(P, npc - t * P)
                xh = apool.tile([P, D], f16, tag="xh")
                if nt < P:
                    nc.vector.memset(xh[:], 0)
                nc.sync.dma_start(xh[:nt], x_half[t * P:t * P + nt, :])
                xt = apool.tile([P, D], f32, tag="xt")
                nc.scalar.copy(xt[:], xh[:])
                xT_ps = apsum.tile([P, P], f32, tag="xT")
                nc.tensor.transpose(xT_ps[:], xt[:], identity[:])
                xTs = apool.tile([P, P], f32, tag="xTs")
                nc.scalar.copy(xTs[:], xT_ps[:])
                T_ps = apsum.tile([P, 4 * D], f32, tag="Tps")
                nc.tensor.matmul(T_ps[:], lhsT=xTs[:], rhs=wbig_t,
                                 start=True, stop=True)
                Tb = apool.tile([P, 4 * D], f32, tag="Tb")
                nc.vector.tensor_add(Tb[:], T_ps[:], bb_t)
                nc.sync.dma_start(T_local[t * P:t * P + nt, :], Tb[:nt])
                q_ps = apsum.tile([P, D], f32, tag="qps")
                nc.tensor.matmul(q_ps[:], lhsT=xTs[:], rhs=wq_t,
                                 start=True, stop=True)
                nc.vector.tensor_add(q_sbuf[:nt, t * D:(t + 1) * D],
                                     q_ps[:nt], bq_t[:nt])

            stkA.close()

            # ================= AllGather node tables =================
            nc.gpsimd.collective_compute(
                "AllGather",
                mybir.AluOpType.bypass,
                replica_groups=[list(range(NCORES))],
                ins=[T_local[:]],
                outs=[T_full[:]],
            )

            # ================= Phase B: edges =================
            stkB = ExitStack()
            bpool = stkB.enter_context(tc.tile_pool(name="b_sb", bufs=4))
            bpsum = stkB.enter_context(tc.tile_pool(name="b_ps", bufs=3, space="PSUM"))
            wpsum = stkB.enter_context(tc.tile_pool(name="win_ps", bufs=2, space="PSUM"))
            for w in range(nwin):
                wr = bpool.tile([P, bpw], i32, tag="wr")
                nc.sync.dma_start(wr[:], wrec[w, :, :])
                km = bpool.tile([P, bpw], i32, tag="km")
                nc.vector.tensor_scalar(km[:], wr[:], KMASK, None,
                                        op0=OP.bitwise_and)
                rli = bpool.tile([P, bpw], i32, tag="rli")
                nc.vector.tensor_scalar(rli[:], wr[:], ROWSHIFT, None,
                                        op0=OP.logical_shift_right)
                rlf = bpool.tile([P, bpw], f32, tag="rlf")
                nc.scalar.copy(rlf[:], rli[:])
                win_ps = wpsum.tile([P, 136], f32, tag="win")
                for b in range(bpw):
                    ktmt = bpool.tile([P, 2 * D], f32, tag="ktmt", bufs=8)
                    nc.gpsimd.indirect_dma_start(
                        out=ktmt[:], out_offset=None,
                        in_=T_full[:],
                        in_offset=bass.IndirectOffsetOnAxis(
                            ap=km[:, b:b + 1], axis=0),
                    )
                    # Sel[e,j] = (rowlocal_e == j)
                    sel = bpool.tile([P, P], f32, tag="sel")
                    nc.vector.tensor_scalar(
                        sel[:], iota_free[:], rlf[:, b:b + 1], None,
                        op0=OP.is_equal)
                    # SelT[j,e] = (j == rowlocal_e)  (TensorE transpose)
                    selT_ps = bpsum.tile([P, P], f32, tag="selTp")
                    nc.tensor.transpose(selT_ps[:], sel[:], identity[:])
                    selT = bpool.tile([P, P], f32, tag="selT")
                    nc.scalar.copy(selT[:], selT_ps[:])
                    # q[dst] for each edge
                    qe_ps = bpsum.tile([P, P], f32, tag="qe")
                    nc.tensor.matmul(qe_ps[:], lhsT=selT[:],
                                     rhs=q_sbuf[:, w * D:(w + 1) * D],
                                     start=True, stop=True)
                    prod = bpool.tile([P, D], f32, tag="prod")
                    nc.vector.tensor_mul(prod[:], ktmt[:][:, 0:D], qe_ps[:])
                    rhs = bpool.tile([P, 136], f32, tag="rhs")
                    nc.vector.tensor_reduce(
                        rhs[:, D:D + H], prod[:].rearrange("p (h c) -> p h c", c=C),
                        axis=mybir.AxisListType.X, op=OP.add)
                    nc.scalar.activation(rhs[:, D:D + H], rhs[:, D:D + H], AF.Exp)
                    nc.vector.tensor_tensor(
                        rhs[:, 0:D].rearrange("p (h c) -> p h c", c=C),
                        ktmt[:][:, D:2 * D].rearrange("p (h c) -> p h c", c=C),
                        rhs[:, D:D + H].rearrange("p (h o) -> p h o", o=1)
                            .to_broadcast([P, H, C]),
                        op=OP.mult)
                    nc.tensor.matmul(win_ps[:], lhsT=sel[:], rhs=rhs[:],
                                     start=(b == 0), stop=(b == bpw - 1))
                nc.scalar.copy(pooled[:, w * 136:(w + 1) * 136], win_ps[:])

            stkB.close()

            # ================= Phase C: aggregate + LN =================
            stkC = ExitStack()
            cpool2 = stkC.enter_context(tc.tile_pool(name="c_sb", bufs=3))
            cpsum = stkC.enter_context(tc.tile_pool(name="c_ps", bufs=2, space="PSUM"))
            for w in range(nwin):
                nt = min(P, npc - w * P)
                num = pooled[:, w * 136:w * 136 + D]
                den = pooled[:, w * 136 + D:w * 136 + D + H]
                denc = cpool2.tile([P, H], f32, tag="denc")
                nc.vector.tensor_scalar_max(denc[:], den, 1e-30)
                inv = cpool2.tile([P, H], f32, tag="inv")
                nc.vector.reciprocal(inv[:], denc[:])
                pn = cpool2.tile([P, D], f32, tag="pn")
                nc.vector.tensor_tensor(
                    pn[:].rearrange("p (h c) -> p h c", c=C),
                    num.rearrange("p (h c) -> p h c", c=C),
                    inv[:].rearrange("p (h o) -> p h o", o=1)
                        .to_broadcast([P, H, C]),
                    op=OP.mult)
                g = cpool2.tile([P, D], f32, tag="g")
                nc.scalar.activation(g[:], pn[:], AF.Gelu)
                gT_ps = cpsum.tile([P, P], f32, tag="gT")
                nc.tensor.transpose(gT_ps[:], g[:], identity[:])
                gTs = cpool2.tile([P, P], f32, tag="gTs")
                nc.scalar.copy(gTs[:], gT_ps[:])
                h_ps = cpsum.tile([P, D], f32, tag="hps")
                nc.tensor.matmul(h_ps[:], lhsT=gTs[:], rhs=wa_t,
                                 start=True, stop=True)
                xh2 = cpool2.tile([P, D], f16, tag="xh2")
                nc.sync.dma_start(xh2[:nt], x_half[w * P:w * P + nt, :])
                o1 = cpool2.tile([P, D], f32, tag="o1")
                nc.vector.tensor_scalar_mul(o1[:], h_ps[:], alpha)
                xt2 = cpool2.tile([P, D], f32, tag="xt2")
                nc.scalar.activation(xt2[:], xh2[:], AF.Copy, scale=1.0 - alpha)
                nc.vector.tensor_add(o1[:], o1[:], xt2[:])
                nc.vector.tensor_add(o1[:], o1[:], baa_t)
                # LayerNorm over features
                mu = cpool2.tile([P, 1], f32, tag="mu")
                nc.vector.tensor_reduce(mu[:], o1[:], axis=mybir.AxisListType.X,
                                        op=OP.add, negate=True)
                nc.vector.tensor_scalar_mul(mu[:], mu[:], 1.0 / D)
                xm = cpool2.tile([P, D], f32, tag="xm")
                nc.vector.tensor_scalar_add(xm[:], o1[:], mu[:, 0:1])
                sq = cpool2.tile([P, D], f32, tag="sq")
                var = cpool2.tile([P, 1], f32, tag="var")
                nc.scalar.activation(sq[:], xm[:], AF.Square,
                                     accum_out=var[:, 0:1])
                std = cpool2.tile([P, 1], f32, tag="std")
                nc.scalar.activation(std[:], var[:], AF.Sqrt, scale=1.0 / D,
                                     bias=LN_EPS)
                rinv = cpool2.tile([P, 1], f32, tag="rinv")
                nc.vector.reciprocal(rinv[:], std[:])
                xn = cpool2.tile([P, D], f32, tag="xn")
                nc.vector.tensor_scalar_mul(xn[:], xm[:], rinv[:, 0:1])
                ot = cpool2.tile([P, D], f32, tag="ot")
                nc.vector.tensor_mul(ot[:], xn[:], gam_t)
                nc.vector.tensor_add(ot[:], ot[:], bet_t)
                oth = cpool2.tile([P, D], f16, tag="oth")
                nc.scalar.copy(oth[:], ot[:])
                nc.sync.dma_start(out[w * P:w * P + nt, :], oth[:nt])
            stkC.close()

    nc.compile()
    return nc


def _make_runner(nc):
    """Build the cached PJRT callable (the same lowering run_bass_kernel_spmd
    uses under axon, minus the per-call retrace and the zeros upload)."""
    import jax
    from jax.sharding import Mesh, PartitionSpec, NamedSharding
    from jax.experimental.shard_map import shard_map
    from concourse import bass2jax, mybir

    bass2jax.install_neuronx_cc_hook()
    partition_name = nc.partition_id_tensor.name if nc.partition_id_tensor else None
    in_names, out_names, out_avals = [], [], []
    for alloc in nc.m.functions[0].allocations:
        if not isinstance(alloc, mybir.MemoryLocationSet):
            continue
        name = alloc.memorylocations[0].name
        if alloc.kind == "ExternalInput":
            if name != partition_name:
                in_names.append(name)
        elif alloc.kind == "ExternalOutput":
            out_names.append(name)
            out_avals.append(jax.core.ShapedArray(
                tuple(alloc.tensor_shape), mybir.dt.np(alloc.dtype)))
    n_params = len(in_names)
    n_outs = len(out_names)
    in_names_all = in_names + out_names
    if partition_name is not None:
        in_names_all.append(partition_name)

    def _body(*args):
        operands = list(args)
        if partition_name is not None:
            operands.append(bass2jax.partition_id_tensor())
        outs = bass2jax._bass_exec_p.bind(
            *operands,
            out_avals=tuple(out_avals),
            in_names=tuple(in_names_all),
            out_names=tuple(out_names),
            lowering_input_output_aliases=(),
            sim_require_finite=True,
            sim_require_nnan=True,
            nc=nc,
        )
        return tuple(outs)

    devices = jax.devices()[:NCORES]
    mesh = Mesh(np.asarray(devices), ("core",))
    sharded = jax.jit(
        shard_map(_body, mesh=mesh,
                  in_specs=(PartitionSpec("core"),) * (n_params + n_outs),
                  out_specs=(PartitionSpec("core"),) * n_outs,
                  check_rep=False),
        keep_unused=True)
    # Output operands only feed the parameter-order check; the kernel writes
    # every element of `out`, so their content is irrelevant -> keep one
    # device-resident buffer alive and pass it every call (no upload).
    sh = NamedSharding(mesh, PartitionSpec("core"))
    dummy = [jax.device_put(
        np.zeros((NCORES * a.shape[0], *a.shape[1:]), a.dtype), sh)
        for a in out_avals]

    def run(in_maps):
        concat_in = [
            np.concatenate([np.asarray(m[name]) for m in in_maps], axis=0)
            for name in in_names]
        outs = sharded(*concat_in, *dummy)
        return {name: np.asarray(o) for name, o in zip(out_names, outs)}

    return run


_CACHE = {}


def _get_compiled(meta):
    key = (meta["n"], meta["npc"], meta["nwin"], meta["bpw"], meta["alpha"])
    if key not in _CACHE:
        nc = _build(meta)
        _CACHE[key] = _make_runner(nc)
    return _CACHE[key]


def kernel(**inputs):
    in_maps, meta = _host_prep(**inputs)
    run = _get_compiled(meta)
    outs = run(in_maps)
    return outs["out"].astype(np.float32)


# revision 12
# speedup vs baseline: 1.7271x; 1.7271x over previous
"""HGT graph update kernel for 8 Trainium2 NeuronCores.

Strategy:
  * Host folds the per-relation projections into node-level weights:
      kt_s = x @ (Wk @ blockdiag(Watt_s)) * prior_s/sqrt(C)   (per head col-block)
      mt_s = x @ (Wm @ blockdiag(Wmsg_s))
    so each edge only needs gathers:  score = <kt_s[src], q[dst]>_per-head,
    msg = mt_s[src].
  * Softmax without the max-subtraction pass (scores are O(1) here; the
    shifted/unshifted softmax are algebraically identical, fp32-safe).
  * All 2E edges are sorted by destination on the host; the 8 cores own
    contiguous 12500-node ranges, so each core completes its own segment
    softmax locally - the only collective is one AllGather of the node
    tables kt/mt (q stays core-local in SBUF).
  * Edge phase: per 128-edge block, one indirect DMA gathers [kt|mt]
    (1024B/edge) from the gathered table; q[dst] is reconstructed with a
    one-hot matmul from SBUF (no DMA); scatter-add into a PSUM window of
    128 consecutive dst nodes via a one-hot matmul.

  Host<->device transport (the axon tunnel, ~50MB/s) dominates, so:
  * x and the output travel as per-row int8 (amax/127 fp16 row scales).
    Tolerance is 2e-2; each int8 leg adds ~6e-3 (HW f32->int8 conversion
    is exact round-to-nearest-even, verified on device).
  * Each edge is packed into ONE int32: kmidx (18b) | rowlocal (8b).
    rowlocal is unpacked on device (shift + int->f32 copy); the one-hot
    selector transpose is done on TensorE instead of shipping a second
    pre-transposed copy of the indices.
  * Weights/biases ship as fp16, biases as single rows broadcast on
    device via a ones-outer-product matmul.
  * The jitted PJRT callable is built once and cached; the output-operand
    required by the bass_exec custom call is a persistent device-resident
    dummy (the kernel writes every output element), so no per-call zeros
    upload.
"""

import sys

if "/opt/trn_rl_repo" not in sys.path:
    sys.path.insert(0, "/opt/trn_rl_repo")
import numpy as np

N, D, H, C = 100000, 128, 8, 16
LN_EPS = 1e-3
NCORES = 8
P = 128
ROWSHIFT = 18                 # kmidx bits 0..17, rowlocal bits 18..25
KMASK = (1 << ROWSHIFT) - 1
DUMMY_ROW = 255               # never matches a 0..127 local row


def _host_prep(x, src0, dst0, src1, dst1, Wk, bk, Wm, bm, Wq, bq, Wa, ba,
               Watt0, Wmsg0, Watt1, Wmsg1, prior0, prior1, skip, gamma, beta):
    """Fold weights, sort edges by dst, build per-core packed index records."""
    f32 = np.float32
    x = np.asarray(x, f32)
    n = x.shape[0]
    npc = n // NCORES            # nodes per core
    nwin = (npc + P - 1) // P    # windows (128-node groups) per core

    def bd(w):  # [H,C,C] -> block-diagonal [D,D]
        out = np.zeros((H * C, H * C), f32)
        for h in range(H):
            out[h * C:(h + 1) * C, h * C:(h + 1) * C] = np.asarray(w[h], f32)
        return out

    scale = 1.0 / np.sqrt(f32(C))
    cs0 = np.repeat(np.asarray(prior0, f32) * scale, C)   # [D] col scale
    cs1 = np.repeat(np.asarray(prior1, f32) * scale, C)
    Wk, bk, Wm, bm = (np.asarray(a, f32) for a in (Wk, bk, Wm, bm))
    Wkt0 = (Wk @ bd(Watt0)) * cs0; bkt0 = (bk @ bd(Watt0)) * cs0
    Wkt1 = (Wk @ bd(Watt1)) * cs1; bkt1 = (bk @ bd(Watt1)) * cs1
    Wmt0 = Wm @ bd(Wmsg0); bmt0 = bm @ bd(Wmsg0)
    Wmt1 = Wm @ bd(Wmsg1); bmt1 = bm @ bd(Wmsg1)
    # T row layout per node: [kt0 | mt0 | kt1 | mt1]  -> viewed as [2n, 256]:
    # row 2s+b = [kt_b | mt_b] of node s.
    Wbig = np.concatenate([Wkt0, Wmt0, Wkt1, Wmt1], axis=1)        # [128, 512]
    bbig = np.concatenate([bkt0, bmt0, bkt1, bmt1])                # [512]

    # ---- edges: sort by dst ----
    e0 = len(np.asarray(src0)); e1 = len(np.asarray(src1))
    src = np.concatenate([np.asarray(src0), np.asarray(src1)]).astype(np.int64)
    dst = np.concatenate([np.asarray(dst0), np.asarray(dst1)]).astype(np.int64)
    eset = np.concatenate([np.zeros(e0, np.int64), np.ones(e1, np.int64)])
    order = np.argsort(dst, kind="stable")
    src, dst, eset = src[order], dst[order], eset[order]
    kmidx = (2 * src + eset).astype(np.int64)      # row into [2n, 256] table

    # per-core, per-window edge ranges: window w of core c covers nodes
    # [c*npc + w*P, next boundary); boundaries are strictly increasing so a
    # single searchsorted over all of them yields every window's edge range.
    win_start = np.asarray(
        [c * npc + w * P for c in range(NCORES) for w in range(nwin)], np.int64)
    starts = np.searchsorted(dst, np.concatenate([win_start, [n]]), side="left")
    cnts = np.diff(starts)                           # edges per window
    bpw = max(1, int((cnts.max() + P - 1) // P))

    # packed record: word = kmidx | rowlocal<<18, block-major within window
    rowloc = dst - np.repeat(win_start, cnts)
    word_all = (kmidx | (rowloc << ROWSHIFT)).astype(np.int32)
    wrec = np.full((NCORES * nwin, bpw * P), DUMMY_ROW << ROWSHIFT, np.int32)
    mask = np.arange(bpw * P)[None, :] < cnts[:, None]
    wrec[mask] = word_all
    # [nwin*P? ] -> per window [bpw, P] -> transpose to [P, bpw]
    wrec = wrec.reshape(NCORES, nwin, bpw, P).transpose(0, 1, 3, 2)
    wrec = np.ascontiguousarray(wrec)

    alpha = float(1.0 / (1.0 + np.exp(-np.float64(np.asarray(skip)))))
    f16 = np.float16
    wall = np.concatenate(
        [Wbig, np.asarray(Wq, f32), np.asarray(Wa, f32)], axis=1).astype(f16)
    crow = np.concatenate(
        [bbig, np.asarray(bq, f32), np.asarray(ba, f32) * f32(alpha),
         np.asarray(gamma, f32), np.asarray(beta, f32)])[None, :].astype(f16)
    consts = dict(wall=wall, crow=crow)
    # per-row int8 quantization of x (device dequantizes with the fp16 scale)
    xs = (np.maximum(np.abs(x).max(axis=1, keepdims=True), 1e-30)
          / f32(127.0)).astype(f16)
    xq = np.clip(np.round(x / xs.astype(f32)), -127, 127).astype(np.int8)
    in_maps = []
    for c in range(NCORES):
        m = dict(consts)
        m["x_q"] = np.ascontiguousarray(xq[c * npc:(c + 1) * npc])
        m["x_s"] = np.ascontiguousarray(xs[c * npc:(c + 1) * npc])
        m["wrec"] = wrec[c]
        in_maps.append(m)
    return in_maps, dict(n=n, npc=npc, nwin=nwin, bpw=bpw, alpha=alpha)


def _build(meta):
    """Build the Bass program (shared by all 8 cores)."""
    import concourse.bass as bass
    import concourse.mybir as mybir
    import concourse.tile as tile
    from concourse.masks import make_identity

    f32 = mybir.dt.float32
    f16 = mybir.dt.float16
    i32 = mybir.dt.int32
    AF = mybir.ActivationFunctionType
    OP = mybir.AluOpType
    n, npc, nwin, bpw = meta["n"], meta["npc"], meta["nwin"], meta["bpw"]
    alpha = meta["alpha"]

    import concourse.bacc as bacc
    nc = bacc.Bacc(trn_type="TRN2", num_devices=NCORES)

    i8 = mybir.dt.int8
    x_q = nc.dram_tensor("x_q", [npc, D], i8, kind="ExternalInput")
    x_s = nc.dram_tensor("x_s", [npc, 1], f16, kind="ExternalInput")
    wrec = nc.dram_tensor("wrec", [nwin, P, bpw], i32, kind="ExternalInput")
    wall = nc.dram_tensor("wall", [D, 6 * D], f16, kind="ExternalInput")
    crow = nc.dram_tensor("crow", [1, 8 * D], f16, kind="ExternalInput")
    out_q = nc.dram_tensor("out_q", [npc, D], i8, kind="ExternalOutput")
    out_s = nc.dram_tensor("out_s", [npc, 1], f16, kind="ExternalOutput")

    from contextlib import ExitStack
    with tile.TileContext(nc, num_cores=NCORES) as tc:
        with (
            tc.tile_pool(name="const", bufs=1) as cpool,
            tc.tile_pool(name="dram", bufs=1, space="DRAM") as dram,
        ):
            # ---- constants ----
            identity = cpool.tile([P, P], f32)
            make_identity(nc, identity[:])
            iota_free = cpool.tile([P, P], f32)
            nc.gpsimd.iota(iota_free[:], pattern=[[1, P]], channel_multiplier=0,
                           allow_small_or_imprecise_dtypes=True)
            ones_row = cpool.tile([1, P], f32)
            nc.vector.memset(ones_row[:], 1.0)
            zero_col = cpool.tile([P, 1], f32)
            nc.vector.memset(zero_col[:], 0.0)
            eps_col = cpool.tile([P, 1], f32)
            nc.vector.memset(eps_col[:], LN_EPS)
            nc.const_aps.aps[(f32, 0.0)] = zero_col[:]
            nc.const_aps.aps[(f32, LN_EPS)] = eps_col[:]

            wall_h = cpool.tile([D, 6 * D], f16)
            nc.sync.dma_start(wall_h[:], wall[:])
            wall_t = cpool.tile([D, 6 * D], f32)
            nc.scalar.copy(wall_t[:], wall_h[:])
            wbig_t = wall_t[:][:, 0:4 * D]
            wq_t = wall_t[:][:, 4 * D:5 * D]
            wa_t = wall_t[:][:, 5 * D:6 * D]

            crow_h = cpool.tile([1, 8 * D], f16)
            nc.sync.dma_start(crow_h[:], crow[:])
            crow_f = cpool.tile([1, 8 * D], f32)
            nc.scalar.copy(crow_f[:], crow_h[:])
            cbrd = cpool.tile([P, 8 * D], f32)
            with tc.tile_pool(name="bc_ps", bufs=2, space="PSUM") as bcps:
                for j in range(2):
                    ps = bcps.tile([P, 4 * D], f32, tag="bc")
                    nc.tensor.matmul(ps[:], lhsT=ones_row[:],
                                     rhs=crow_f[:][:, j * 4 * D:(j + 1) * 4 * D],
                                     start=True, stop=True)
                    nc.scalar.copy(cbrd[:, j * 4 * D:(j + 1) * 4 * D], ps[:])
            bb_t = cbrd[:][:, 0:4 * D]
            bq_t = cbrd[:][:, 4 * D:5 * D]
            baa_t = cbrd[:][:, 5 * D:6 * D]
            gam_t = cbrd[:][:, 6 * D:7 * D]
            bet_t = cbrd[:][:, 7 * D:8 * D]

            # persistent SBUF state
            q_sbuf = cpool.tile([P, nwin * D], f32)
            nc.gpsimd.memset(q_sbuf[:], 0)
            pooled = cpool.tile([P, nwin * 136], f32)

            T_local = dram.tile([npc, 4 * D], f32)
            T_full = dram.tile([2 * n, 2 * D], f32)

            # ================= Phase A: projections =================
            stkA = ExitStack()
            apool = stkA.enter_context(tc.tile_pool(name="a_sb", bufs=3))
            apsum = stkA.enter_context(tc.tile_pool(name="a_ps", bufs=2, space="PSUM"))
            for t in range(nwin):
                nt = min(P, npc - t * P)
                xq8 = apool.tile([P, D], i8, tag="xq8")
                xsh = apool.tile([P, 1], f16, tag="xsh")
                if nt < P:
                    nc.vector.memset(xq8[:], 0)
                    nc.vector.memset(xsh[:], 0)
                nc.sync.dma_start(xq8[:nt], x_q[t * P:t * P + nt, :])
                nc.sync.dma_start(xsh[:nt], x_s[t * P:t * P + nt, :])
                xsf = apool.tile([P, 1], f32, tag="xsf")
                nc.scalar.copy(xsf[:], xsh[:])
                xt = apool.tile([P, D], f32, tag="xt")
                nc.scalar.activation(xt[:], xq8[:], AF.Copy, scale=xsf[:, 0:1])
                xT_ps = apsum.tile([P, P], f32, tag="xT")
                nc.tensor.transpose(xT_ps[:], xt[:], identity[:])
                xTs = apool.tile([P, P], f32, tag="xTs")
                nc.scalar.copy(xTs[:], xT_ps[:])
                T_ps = apsum.tile([P, 4 * D], f32, tag="Tps")
                nc.tensor.matmul(T_ps[:], lhsT=xTs[:], rhs=wbig_t,
                                 start=True, stop=True)
                Tb = apool.tile([P, 4 * D], f32, tag="Tb")
                nc.vector.tensor_add(Tb[:], T_ps[:], bb_t)
                nc.sync.dma_start(T_local[t * P:t * P + nt, :], Tb[:nt])
                q_ps = apsum.tile([P, D], f32, tag="qps")
                nc.tensor.matmul(q_ps[:], lhsT=xTs[:], rhs=wq_t,
                                 start=True, stop=True)
                nc.vector.tensor_add(q_sbuf[:nt, t * D:(t + 1) * D],
                                     q_ps[:nt], bq_t[:nt])

            stkA.close()

            # ================= AllGather node tables =================
            nc.gpsimd.collective_compute(
                "AllGather",
                mybir.AluOpType.bypass,
                replica_groups=[list(range(NCORES))],
                ins=[T_local[:]],
                outs=[T_full[:]],
            )

            # ================= Phase B: edges =================
            stkB = ExitStack()
            bpool = stkB.enter_context(tc.tile_pool(name="b_sb", bufs=4))
            bpsum = stkB.enter_context(tc.tile_pool(name="b_ps", bufs=3, space="PSUM"))
            wpsum = stkB.enter_context(tc.tile_pool(name="win_ps", bufs=2, space="PSUM"))
            for w in range(nwin):
                wr = bpool.tile([P, bpw], i32, tag="wr")
                nc.sync.dma_start(wr[:], wrec[w, :, :])
                km = bpool.tile([P, bpw], i32, tag="km")
                nc.vector.tensor_scalar(km[:], wr[:], KMASK, None,
                                        op0=OP.bitwise_and)
                rli = bpool.tile([P, bpw], i32, tag="rli")
                nc.vector.tensor_scalar(rli[:], wr[:], ROWSHIFT, None,
                                        op0=OP.logical_shift_right)
                rlf = bpool.tile([P, bpw], f32, tag="rlf")
                nc.scalar.copy(rlf[:], rli[:])
                win_ps = wpsum.tile([P, 136], f32, tag="win")
                for b in range(bpw):
                    ktmt = bpool.tile([P, 2 * D], f32, tag="ktmt", bufs=8)
                    nc.gpsimd.indirect_dma_start(
                        out=ktmt[:], out_offset=None,
                        in_=T_full[:],
                        in_offset=bass.IndirectOffsetOnAxis(
                            ap=km[:, b:b + 1], axis=0),
                    )
                    # Sel[e,j] = (rowlocal_e == j)
                    sel = bpool.tile([P, P], f32, tag="sel")
                    nc.vector.tensor_scalar(
                        sel[:], iota_free[:], rlf[:, b:b + 1], None,
                        op0=OP.is_equal)
                    # SelT[j,e] = (j == rowlocal_e)  (TensorE transpose)
                    selT_ps = bpsum.tile([P, P], f32, tag="selTp")
                    nc.tensor.transpose(selT_ps[:], sel[:], identity[:])
                    selT = bpool.tile([P, P], f32, tag="selT")
                    nc.scalar.copy(selT[:], selT_ps[:])
                    # q[dst] for each edge
                    qe_ps = bpsum.tile([P, P], f32, tag="qe")
                    nc.tensor.matmul(qe_ps[:], lhsT=selT[:],
                                     rhs=q_sbuf[:, w * D:(w + 1) * D],
                                     start=True, stop=True)
                    prod = bpool.tile([P, D], f32, tag="prod")
                    nc.vector.tensor_mul(prod[:], ktmt[:][:, 0:D], qe_ps[:])
                    rhs = bpool.tile([P, 136], f32, tag="rhs")
                    nc.vector.tensor_reduce(
                        rhs[:, D:D + H], prod[:].rearrange("p (h c) -> p h c", c=C),
                        axis=mybir.AxisListType.X, op=OP.add)
                    nc.scalar.activation(rhs[:, D:D + H], rhs[:, D:D + H], AF.Exp)
                    nc.vector.tensor_tensor(
                        rhs[:, 0:D].rearrange("p (h c) -> p h c", c=C),
                        ktmt[:][:, D:2 * D].rearrange("p (h c) -> p h c", c=C),
                        rhs[:, D:D + H].rearrange("p (h o) -> p h o", o=1)
                            .to_broadcast([P, H, C]),
                        op=OP.mult)
                    nc.tensor.matmul(win_ps[:], lhsT=sel[:], rhs=rhs[:],
                                     start=(b == 0), stop=(b == bpw - 1))
                nc.scalar.copy(pooled[:, w * 136:(w + 1) * 136], win_ps[:])

            stkB.close()

            # ================= Phase C: aggregate + LN =================
            stkC = ExitStack()
            cpool2 = stkC.enter_context(tc.tile_pool(name="c_sb", bufs=3))
            cpsum = stkC.enter_context(tc.tile_pool(name="c_ps", bufs=2, space="PSUM"))
            for w in range(nwin):
                nt = min(P, npc - w * P)
                num = pooled[:, w * 136:w * 136 + D]
                den = pooled[:, w * 136 + D:w * 136 + D + H]
                denc = cpool2.tile([P, H], f32, tag="denc")
                nc.vector.tensor_scalar_max(denc[:], den, 1e-30)
                inv = cpool2.tile([P, H], f32, tag="inv")
                nc.vector.reciprocal(inv[:], denc[:])
                pn = cpool2.tile([P, D], f32, tag="pn")
                nc.vector.tensor_tensor(
                    pn[:].rearrange("p (h c) -> p h c", c=C),
                    num.rearrange("p (h c) -> p h c", c=C),
                    inv[:].rearrange("p (h o) -> p h o", o=1)
                        .to_broadcast([P, H, C]),
                    op=OP.mult)
                g = cpool2.tile([P, D], f32, tag="g")
                nc.scalar.activation(g[:], pn[:], AF.Gelu)
                gT_ps = cpsum.tile([P, P], f32, tag="gT")
                nc.tensor.transpose(gT_ps[:], g[:], identity[:])
                gTs = cpool2.tile([P, P], f32, tag="gTs")
                nc.scalar.copy(gTs[:], gT_ps[:])
                h_ps = cpsum.tile([P, D], f32, tag="hps")
                nc.tensor.matmul(h_ps[:], lhsT=gTs[:], rhs=wa_t,
                                 start=True, stop=True)
                xq2 = cpool2.tile([P, D], i8, tag="xq2")
                nc.sync.dma_start(xq2[:nt], x_q[w * P:w * P + nt, :])
                xsh2 = cpool2.tile([P, 1], f16, tag="xsh2")
                nc.sync.dma_start(xsh2[:nt], x_s[w * P:w * P + nt, :])
                xsa = cpool2.tile([P, 1], f32, tag="xsa")
                nc.scalar.activation(xsa[:], xsh2[:], AF.Copy, scale=1.0 - alpha)
                o1 = cpool2.tile([P, D], f32, tag="o1")
                nc.vector.tensor_scalar_mul(o1[:], h_ps[:], alpha)
                xt2 = cpool2.tile([P, D], f32, tag="xt2")
                nc.scalar.activation(xt2[:], xq2[:], AF.Copy, scale=xsa[:, 0:1])
                nc.vector.tensor_add(o1[:], o1[:], xt2[:])
                nc.vector.tensor_add(o1[:], o1[:], baa_t)
                # LayerNorm over features
                mu = cpool2.tile([P, 1], f32, tag="mu")
                nc.vector.tensor_reduce(mu[:], o1[:], axis=mybir.AxisListType.X,
                                        op=OP.add, negate=True)
                nc.vector.tensor_scalar_mul(mu[:], mu[:], 1.0 / D)
                xm = cpool2.tile([P, D], f32, tag="xm")
                nc.vector.tensor_scalar_add(xm[:], o1[:], mu[:, 0:1])
                sq = cpool2.tile([P, D], f32, tag="sq")
                var = cpool2.tile([P, 1], f32, tag="var")
                nc.scalar.activation(sq[:], xm[:], AF.Square,
                                     accum_out=var[:, 0:1])
                std = cpool2.tile([P, 1], f32, tag="std")
                nc.scalar.activation(std[:], var[:], AF.Sqrt, scale=1.0 / D,
                                     bias=LN_EPS)
                rinv = cpool2.tile([P, 1], f32, tag="rinv")
                nc.vector.reciprocal(rinv[:], std[:])
                xn = cpool2.tile([P, D], f32, tag="xn")
                nc.vector.tensor_scalar_mul(xn[:], xm[:], rinv[:, 0:1])
                ot = cpool2.tile([P, D], f32, tag="ot")
                nc.vector.tensor_mul(ot[:], xn[:], gam_t)
                nc.vector.tensor_add(ot[:], ot[:], bet_t)
                # per-row int8 quantization: q = round(ot * 127/amax)
                oab = cpool2.tile([P, D], f32, tag="oab")
                nc.scalar.activation(oab[:], ot[:], AF.Abs)
                oam = cpool2.tile([P, 1], f32, tag="oam")
                nc.vector.tensor_reduce(oam[:], oab[:], axis=mybir.AxisListType.X,
                                        op=OP.max)
                osc = cpool2.tile([P, 1], f32, tag="osc")
                nc.vector.tensor_scalar_max(osc[:], oam[:], 1e-30)
                nc.vector.tensor_scalar_mul(osc[:], osc[:], 1.0 / 127.0)
                oinv = cpool2.tile([P, 1], f32, tag="oinv")
                nc.vector.reciprocal(oinv[:], osc[:])
                oq = cpool2.tile([P, D], i8, tag="oq")
                nc.scalar.activation(oq[:], ot[:], AF.Copy, scale=oinv[:, 0:1])
                osh = cpool2.tile([P, 1], f16, tag="osh")
                nc.scalar.copy(osh[:], osc[:])
                nc.sync.dma_start(out_q[w * P:w * P + nt, :], oq[:nt])
                nc.sync.dma_start(out_s[w * P:w * P + nt, :], osh[:nt])
            stkC.close()

    nc.compile()
    return nc


def _make_runner(nc):
    """Build the cached PJRT callable (the same lowering run_bass_kernel_spmd
    uses under axon, minus the per-call retrace and the zeros upload)."""
    import jax
    from jax.sharding import Mesh, PartitionSpec, NamedSharding
    from jax.experimental.shard_map import shard_map
    from concourse import bass2jax, mybir

    bass2jax.install_neuronx_cc_hook()
    partition_name = nc.partition_id_tensor.name if nc.partition_id_tensor else None
    in_names, out_names, out_avals = [], [], []
    for alloc in nc.m.functions[0].allocations:
        if not isinstance(alloc, mybir.MemoryLocationSet):
            continue
        name = alloc.memorylocations[0].name
        if alloc.kind == "ExternalInput":
            if name != partition_name:
                in_names.append(name)
        elif alloc.kind == "ExternalOutput":
            out_names.append(name)
            out_avals.append(jax.core.ShapedArray(
                tuple(alloc.tensor_shape), mybir.dt.np(alloc.dtype)))
    n_params = len(in_names)
    n_outs = len(out_names)
    in_names_all = in_names + out_names
    if partition_name is not None:
        in_names_all.append(partition_name)

    def _body(*args):
        operands = list(args)
        if partition_name is not None:
            operands.append(bass2jax.partition_id_tensor())
        outs = bass2jax._bass_exec_p.bind(
            *operands,
            out_avals=tuple(out_avals),
            in_names=tuple(in_names_all),
            out_names=tuple(out_names),
            lowering_input_output_aliases=(),
            sim_require_finite=True,
            sim_require_nnan=True,
            nc=nc,
        )
        return tuple(outs)

    devices = jax.devices()[:NCORES]
    mesh = Mesh(np.asarray(devices), ("core",))
    sharded = jax.jit(
        shard_map(_body, mesh=mesh,
                  in_specs=(PartitionSpec("core"),) * (n_params + n_outs),
                  out_specs=(PartitionSpec("core"),) * n_outs,
                  check_rep=False),
        keep_unused=True)
    # Output operands only feed the parameter-order check; the kernel writes
    # every element of `out`, so their content is irrelevant -> keep one
    # device-resident buffer alive and pass it every call (no upload).
    sh = NamedSharding(mesh, PartitionSpec("core"))
    dummy = [jax.device_put(
        np.zeros((NCORES * a.shape[0], *a.shape[1:]), a.dtype), sh)
        for a in out_avals]

    def run(in_maps):
        concat_in = [
            np.concatenate([np.asarray(m[name]) for m in in_maps], axis=0)
            for name in in_names]
        outs = sharded(*concat_in, *dummy)
        return {name: np.asarray(o) for name, o in zip(out_names, outs)}

    return run


_CACHE = {}


def _get_compiled(meta):
    key = (meta["n"], meta["npc"], meta["nwin"], meta["bpw"], meta["alpha"])
    if key not in _CACHE:
        nc = _build(meta)
        _CACHE[key] = _make_runner(nc)
    return _CACHE[key]


def kernel(**inputs):
    in_maps, meta = _host_prep(**inputs)
    run = _get_compiled(meta)
    outs = run(in_maps)
    return outs["out_q"].astype(np.float32) * outs["out_s"].astype(np.float32)


# revision 13
# speedup vs baseline: 1.9412x; 1.1240x over previous
"""HGT graph update kernel for 8 Trainium2 NeuronCores.

Strategy:
  * Host folds the per-relation projections into node-level weights:
      kt_s = x @ (Wk @ blockdiag(Watt_s)) * prior_s/sqrt(C)   (per head col-block)
      mt_s = x @ (Wm @ blockdiag(Wmsg_s))
    so each edge only needs gathers:  score = <kt_s[src], q[dst]>_per-head,
    msg = mt_s[src].
  * Softmax without the max-subtraction pass (scores are O(1) here; the
    shifted/unshifted softmax are algebraically identical, fp32-safe).
  * All 2E edges are sorted by destination on the host; the 8 cores own
    contiguous 12500-node ranges, so each core completes its own segment
    softmax locally - the only collective is one AllGather of the node
    tables kt/mt (q stays core-local in SBUF).
  * Edge phase: per 128-edge block, one indirect DMA gathers [kt|mt]
    (1024B/edge) from the gathered table; q[dst] is reconstructed with a
    one-hot matmul from SBUF (no DMA); scatter-add into a PSUM window of
    128 consecutive dst nodes via a one-hot matmul.

  Host<->device transport (the axon tunnel, ~50MB/s) dominates, so:
  * x and the output travel as per-row int8 (amax/127 fp16 row scales).
    Tolerance is 2e-2; each int8 leg adds ~6e-3 (HW f32->int8 conversion
    is exact round-to-nearest-even, verified on device).
  * Each edge is packed into ONE int32: kmidx (18b) | rowlocal (8b).
    rowlocal is unpacked on device (shift + int->f32 copy); the one-hot
    selector transpose is done on TensorE instead of shipping a second
    pre-transposed copy of the indices.
  * Weights/biases ship as fp16, biases as single rows broadcast on
    device via a ones-outer-product matmul.
  * The jitted PJRT callable is built once and cached; the output-operand
    required by the bass_exec custom call is a persistent device-resident
    dummy (the kernel writes every output element), so no per-call zeros
    upload.
"""

import sys

if "/opt/trn_rl_repo" not in sys.path:
    sys.path.insert(0, "/opt/trn_rl_repo")
import numpy as np

N, D, H, C = 100000, 128, 8, 16
LN_EPS = 1e-3
NCORES = 8
P = 128
ROWSHIFT = 18                 # kmidx bits 0..17, rowlocal bits 18..25
KMASK = (1 << ROWSHIFT) - 1
DUMMY_ROW = 255               # never matches a 0..127 local row


def _host_prep(x, src0, dst0, src1, dst1, Wk, bk, Wm, bm, Wq, bq, Wa, ba,
               Watt0, Wmsg0, Watt1, Wmsg1, prior0, prior1, skip, gamma, beta):
    """Fold weights, sort edges by dst, build per-core packed index records."""
    f32 = np.float32
    x = np.asarray(x, f32)
    n = x.shape[0]
    npc = n // NCORES            # nodes per core
    nwin = (npc + P - 1) // P    # windows (128-node groups) per core

    def bd(w):  # [H,C,C] -> block-diagonal [D,D]
        out = np.zeros((H * C, H * C), f32)
        for h in range(H):
            out[h * C:(h + 1) * C, h * C:(h + 1) * C] = np.asarray(w[h], f32)
        return out

    scale = 1.0 / np.sqrt(f32(C))
    cs0 = np.repeat(np.asarray(prior0, f32) * scale, C)   # [D] col scale
    cs1 = np.repeat(np.asarray(prior1, f32) * scale, C)
    Wk, bk, Wm, bm = (np.asarray(a, f32) for a in (Wk, bk, Wm, bm))
    Wkt0 = (Wk @ bd(Watt0)) * cs0; bkt0 = (bk @ bd(Watt0)) * cs0
    Wkt1 = (Wk @ bd(Watt1)) * cs1; bkt1 = (bk @ bd(Watt1)) * cs1
    Wmt0 = Wm @ bd(Wmsg0); bmt0 = bm @ bd(Wmsg0)
    Wmt1 = Wm @ bd(Wmsg1); bmt1 = bm @ bd(Wmsg1)
    # T row layout per node: [kt0 | mt0 | kt1 | mt1]  -> viewed as [2n, 256]:
    # row 2s+b = [kt_b | mt_b] of node s.
    Wbig = np.concatenate([Wkt0, Wmt0, Wkt1, Wmt1], axis=1)        # [128, 512]
    bbig = np.concatenate([bkt0, bmt0, bkt1, bmt1])                # [512]

    # ---- edges: sort by dst ----
    e0 = len(np.asarray(src0)); e1 = len(np.asarray(src1))
    src = np.concatenate([np.asarray(src0), np.asarray(src1)]).astype(np.int64)
    dst = np.concatenate([np.asarray(dst0), np.asarray(dst1)]).astype(np.int64)
    eset = np.concatenate([np.zeros(e0, np.int64), np.ones(e1, np.int64)])
    order = np.argsort(dst, kind="stable")
    src, dst, eset = src[order], dst[order], eset[order]
    kmidx = (2 * src + eset).astype(np.int64)      # row into [2n, 256] table

    # per-core, per-window edge ranges: window w of core c covers nodes
    # [c*npc + w*P, next boundary); boundaries are strictly increasing so a
    # single searchsorted over all of them yields every window's edge range.
    win_start = np.asarray(
        [c * npc + w * P for c in range(NCORES) for w in range(nwin)], np.int64)
    starts = np.searchsorted(dst, np.concatenate([win_start, [n]]), side="left")
    cnts = np.diff(starts)                           # edges per window
    bpw = max(1, int((cnts.max() + P - 1) // P))

    # packed record: word = kmidx | rowlocal<<18, block-major within window
    rowloc = dst - np.repeat(win_start, cnts)
    word_all = (kmidx | (rowloc << ROWSHIFT)).astype(np.int32)
    wrec = np.full((NCORES * nwin, bpw * P), DUMMY_ROW << ROWSHIFT, np.int32)
    mask = np.arange(bpw * P)[None, :] < cnts[:, None]
    wrec[mask] = word_all
    # [nwin*P? ] -> per window [bpw, P] -> transpose to [P, bpw]
    wrec = wrec.reshape(NCORES, nwin, bpw, P).transpose(0, 1, 3, 2)
    wrec = np.ascontiguousarray(wrec)

    alpha = float(1.0 / (1.0 + np.exp(-np.float64(np.asarray(skip)))))
    f16 = np.float16
    wall = np.concatenate(
        [Wbig, np.asarray(Wq, f32), np.asarray(Wa, f32)], axis=1).astype(f16)
    crow = np.concatenate(
        [bbig, np.asarray(bq, f32), np.asarray(ba, f32) * f32(alpha),
         np.asarray(gamma, f32), np.asarray(beta, f32)])[None, :].astype(f16)
    consts = dict(wall=wall, crow=crow)
    # per-row int8 quantization of x (device dequantizes with the fp16 scale)
    xs = (np.maximum(np.abs(x).max(axis=1, keepdims=True), 1e-30)
          / f32(127.0)).astype(f16)
    xq = np.clip(np.round(x / xs.astype(f32)), -127, 127).astype(np.int8)
    in_maps = []
    for c in range(NCORES):
        m = dict(consts)
        m["x_q"] = np.ascontiguousarray(xq[c * npc:(c + 1) * npc])
        m["x_s"] = np.ascontiguousarray(xs[c * npc:(c + 1) * npc])
        m["wrec"] = wrec[c]
        in_maps.append(m)
    return in_maps, dict(n=n, npc=npc, nwin=nwin, bpw=bpw, alpha=alpha)


def _build(meta):
    """Build the Bass program (shared by all 8 cores)."""
    import concourse.bass as bass
    import concourse.mybir as mybir
    import concourse.tile as tile
    from concourse.masks import make_identity

    f32 = mybir.dt.float32
    f16 = mybir.dt.float16
    i32 = mybir.dt.int32
    AF = mybir.ActivationFunctionType
    OP = mybir.AluOpType
    n, npc, nwin, bpw = meta["n"], meta["npc"], meta["nwin"], meta["bpw"]
    alpha = meta["alpha"]

    import concourse.bacc as bacc
    nc = bacc.Bacc(trn_type="TRN2", num_devices=NCORES)

    i8 = mybir.dt.int8
    x_q = nc.dram_tensor("x_q", [npc, D], i8, kind="ExternalInput")
    x_s = nc.dram_tensor("x_s", [npc, 1], f16, kind="ExternalInput")
    wrec = nc.dram_tensor("wrec", [nwin, P, bpw], i32, kind="ExternalInput")
    wall = nc.dram_tensor("wall", [D, 6 * D], f16, kind="ExternalInput")
    crow = nc.dram_tensor("crow", [1, 8 * D], f16, kind="ExternalInput")
    out_q = nc.dram_tensor("out_q", [npc, D], i8, kind="ExternalOutput")
    out_s = nc.dram_tensor("out_s", [npc, 1], f16, kind="ExternalOutput")

    from contextlib import ExitStack
    with tile.TileContext(nc, num_cores=NCORES) as tc:
        with (
            tc.tile_pool(name="const", bufs=1) as cpool,
            tc.tile_pool(name="dram", bufs=1, space="DRAM") as dram,
        ):
            # ---- constants ----
            identity = cpool.tile([P, P], f32)
            make_identity(nc, identity[:])
            iota_free = cpool.tile([P, P], f32)
            nc.gpsimd.iota(iota_free[:], pattern=[[1, P]], channel_multiplier=0,
                           allow_small_or_imprecise_dtypes=True)
            ones_row = cpool.tile([1, P], f32)
            nc.vector.memset(ones_row[:], 1.0)
            zero_col = cpool.tile([P, 1], f32)
            nc.vector.memset(zero_col[:], 0.0)
            eps_col = cpool.tile([P, 1], f32)
            nc.vector.memset(eps_col[:], LN_EPS)
            nc.const_aps.aps[(f32, 0.0)] = zero_col[:]
            nc.const_aps.aps[(f32, LN_EPS)] = eps_col[:]

            wall_h = cpool.tile([D, 6 * D], f16)
            nc.sync.dma_start(wall_h[:], wall[:])
            wall_t = cpool.tile([D, 6 * D], f32)
            nc.scalar.copy(wall_t[:], wall_h[:])
            wbig_t = wall_t[:][:, 0:4 * D]
            wq_t = wall_t[:][:, 4 * D:5 * D]
            wa_t = wall_t[:][:, 5 * D:6 * D]

            crow_h = cpool.tile([1, 8 * D], f16)
            nc.sync.dma_start(crow_h[:], crow[:])
            crow_f = cpool.tile([1, 8 * D], f32)
            nc.scalar.copy(crow_f[:], crow_h[:])
            cbrd = cpool.tile([P, 8 * D], f32)
            with tc.tile_pool(name="bc_ps", bufs=2, space="PSUM") as bcps:
                for j in range(2):
                    ps = bcps.tile([P, 4 * D], f32, tag="bc")
                    nc.tensor.matmul(ps[:], lhsT=ones_row[:],
                                     rhs=crow_f[:][:, j * 4 * D:(j + 1) * 4 * D],
                                     start=True, stop=True)
                    nc.scalar.copy(cbrd[:, j * 4 * D:(j + 1) * 4 * D], ps[:])
            bb_t = cbrd[:][:, 0:4 * D]
            bq_t = cbrd[:][:, 4 * D:5 * D]
            baa_t = cbrd[:][:, 5 * D:6 * D]
            gam_t = cbrd[:][:, 6 * D:7 * D]
            bet_t = cbrd[:][:, 7 * D:8 * D]

            # persistent SBUF state
            q_sbuf = cpool.tile([P, nwin * D], f32)
            nc.gpsimd.memset(q_sbuf[:], 0)
            pooled = cpool.tile([P, nwin * 136], f32)

            T_local = dram.tile([npc, 4 * D], f32)
            T_full = dram.tile([2 * n, 2 * D], f32)

            # ================= Phase A: projections =================
            stkA = ExitStack()
            apool = stkA.enter_context(tc.tile_pool(name="a_sb", bufs=3))
            apsum = stkA.enter_context(tc.tile_pool(name="a_ps", bufs=2, space="PSUM"))
            for t in range(nwin):
                nt = min(P, npc - t * P)
                xq8 = apool.tile([P, D], i8, tag="xq8")
                xsh = apool.tile([P, 1], f16, tag="xsh")
                if nt < P:
                    nc.vector.memset(xq8[:], 0)
                    nc.vector.memset(xsh[:], 0)
                nc.sync.dma_start(xq8[:nt], x_q[t * P:t * P + nt, :])
                nc.sync.dma_start(xsh[:nt], x_s[t * P:t * P + nt, :])
                xsf = apool.tile([P, 1], f32, tag="xsf")
                nc.scalar.copy(xsf[:], xsh[:])
                xt = apool.tile([P, D], f32, tag="xt")
                nc.scalar.activation(xt[:], xq8[:], AF.Copy, scale=xsf[:, 0:1])
                xT_ps = apsum.tile([P, P], f32, tag="xT")
                nc.tensor.transpose(xT_ps[:], xt[:], identity[:])
                xTs = apool.tile([P, P], f32, tag="xTs")
                nc.scalar.copy(xTs[:], xT_ps[:])
                T_ps = apsum.tile([P, 4 * D], f32, tag="Tps")
                nc.tensor.matmul(T_ps[:], lhsT=xTs[:], rhs=wbig_t,
                                 start=True, stop=True)
                Tb = apool.tile([P, 4 * D], f32, tag="Tb")
                nc.vector.tensor_add(Tb[:], T_ps[:], bb_t)
                nc.sync.dma_start(T_local[t * P:t * P + nt, :], Tb[:nt])
                q_ps = apsum.tile([P, D], f32, tag="qps")
                nc.tensor.matmul(q_ps[:], lhsT=xTs[:], rhs=wq_t,
                                 start=True, stop=True)
                nc.vector.tensor_add(q_sbuf[:nt, t * D:(t + 1) * D],
                                     q_ps[:nt], bq_t[:nt])

            stkA.close()

            # ================= AllGather node tables =================
            nc.gpsimd.collective_compute(
                "AllGather",
                mybir.AluOpType.bypass,
                replica_groups=[list(range(NCORES))],
                ins=[T_local[:]],
                outs=[T_full[:]],
            )

            # ================= Phase B: edges =================
            stkB = ExitStack()
            bpool = stkB.enter_context(tc.tile_pool(name="b_sb", bufs=4))
            bpsum = stkB.enter_context(tc.tile_pool(name="b_ps", bufs=3, space="PSUM"))
            wpsum = stkB.enter_context(tc.tile_pool(name="win_ps", bufs=2, space="PSUM"))
            for w in range(nwin):
                wr = bpool.tile([P, bpw], i32, tag="wr")
                nc.sync.dma_start(wr[:], wrec[w, :, :])
                km = bpool.tile([P, bpw], i32, tag="km")
                nc.vector.tensor_scalar(km[:], wr[:], KMASK, None,
                                        op0=OP.bitwise_and)
                rli = bpool.tile([P, bpw], i32, tag="rli")
                nc.vector.tensor_scalar(rli[:], wr[:], ROWSHIFT, None,
                                        op0=OP.logical_shift_right)
                rlf = bpool.tile([P, bpw], f32, tag="rlf")
                nc.scalar.copy(rlf[:], rli[:])
                win_ps = wpsum.tile([P, 136], f32, tag="win")
                for b in range(bpw):
                    ktmt = bpool.tile([P, 2 * D], f32, tag="ktmt", bufs=8)
                    nc.gpsimd.indirect_dma_start(
                        out=ktmt[:], out_offset=None,
                        in_=T_full[:],
                        in_offset=bass.IndirectOffsetOnAxis(
                            ap=km[:, b:b + 1], axis=0),
                    )
                    # Sel[e,j] = (rowlocal_e == j)
                    sel = bpool.tile([P, P], f32, tag="sel")
                    nc.vector.tensor_scalar(
                        sel[:], iota_free[:], rlf[:, b:b + 1], None,
                        op0=OP.is_equal)
                    # SelT[j,e] = (j == rowlocal_e)  (TensorE transpose)
                    selT_ps = bpsum.tile([P, P], f32, tag="selTp")
                    nc.tensor.transpose(selT_ps[:], sel[:], identity[:])
                    selT = bpool.tile([P, P], f32, tag="selT")
                    nc.scalar.copy(selT[:], selT_ps[:])
                    # q[dst] for each edge
                    qe_ps = bpsum.tile([P, P], f32, tag="qe")
                    nc.tensor.matmul(qe_ps[:], lhsT=selT[:],
                                     rhs=q_sbuf[:, w * D:(w + 1) * D],
                                     start=True, stop=True)
                    prod = bpool.tile([P, D], f32, tag="prod")
                    nc.vector.tensor_mul(prod[:], ktmt[:][:, 0:D], qe_ps[:])
                    rhs = bpool.tile([P, 136], f32, tag="rhs")
                    nc.vector.tensor_reduce(
                        rhs[:, D:D + H], prod[:].rearrange("p (h c) -> p h c", c=C),
                        axis=mybir.AxisListType.X, op=OP.add)
                    nc.scalar.activation(rhs[:, D:D + H], rhs[:, D:D + H], AF.Exp)
                    nc.vector.tensor_tensor(
                        rhs[:, 0:D].rearrange("p (h c) -> p h c", c=C),
                        ktmt[:][:, D:2 * D].rearrange("p (h c) -> p h c", c=C),
                        rhs[:, D:D + H].rearrange("p (h o) -> p h o", o=1)
                            .to_broadcast([P, H, C]),
                        op=OP.mult)
                    nc.tensor.matmul(win_ps[:], lhsT=sel[:], rhs=rhs[:],
                                     start=(b == 0), stop=(b == bpw - 1))
                nc.scalar.copy(pooled[:, w * 136:(w + 1) * 136], win_ps[:])

            stkB.close()

            # ================= Phase C: aggregate + LN =================
            stkC = ExitStack()
            cpool2 = stkC.enter_context(tc.tile_pool(name="c_sb", bufs=3))
            cpsum = stkC.enter_context(tc.tile_pool(name="c_ps", bufs=2, space="PSUM"))
            for w in range(nwin):
                nt = min(P, npc - w * P)
                num = pooled[:, w * 136:w * 136 + D]
                den = pooled[:, w * 136 + D:w * 136 + D + H]
                denc = cpool2.tile([P, H], f32, tag="denc")
                nc.vector.tensor_scalar_max(denc[:], den, 1e-30)
                inv = cpool2.tile([P, H], f32, tag="inv")
                nc.vector.reciprocal(inv[:], denc[:])
                pn = cpool2.tile([P, D], f32, tag="pn")
                nc.vector.tensor_tensor(
                    pn[:].rearrange("p (h c) -> p h c", c=C),
                    num.rearrange("p (h c) -> p h c", c=C),
                    inv[:].rearrange("p (h o) -> p h o", o=1)
                        .to_broadcast([P, H, C]),
                    op=OP.mult)
                g = cpool2.tile([P, D], f32, tag="g")
                nc.scalar.activation(g[:], pn[:], AF.Gelu)
                gT_ps = cpsum.tile([P, P], f32, tag="gT")
                nc.tensor.transpose(gT_ps[:], g[:], identity[:])
                gTs = cpool2.tile([P, P], f32, tag="gTs")
                nc.scalar.copy(gTs[:], gT_ps[:])
                h_ps = cpsum.tile([P, D], f32, tag="hps")
                nc.tensor.matmul(h_ps[:], lhsT=gTs[:], rhs=wa_t,
                                 start=True, stop=True)
                xq2 = cpool2.tile([P, D], i8, tag="xq2")
                nc.sync.dma_start(xq2[:nt], x_q[w * P:w * P + nt, :])
                xsh2 = cpool2.tile([P, 1], f16, tag="xsh2")
                nc.sync.dma_start(xsh2[:nt], x_s[w * P:w * P + nt, :])
                xsa = cpool2.tile([P, 1], f32, tag="xsa")
                nc.scalar.activation(xsa[:], xsh2[:], AF.Copy, scale=1.0 - alpha)
                o1 = cpool2.tile([P, D], f32, tag="o1")
                nc.vector.tensor_scalar_mul(o1[:], h_ps[:], alpha)
                xt2 = cpool2.tile([P, D], f32, tag="xt2")
                nc.scalar.activation(xt2[:], xq2[:], AF.Copy, scale=xsa[:, 0:1])
                nc.vector.tensor_add(o1[:], o1[:], xt2[:])
                nc.vector.tensor_add(o1[:], o1[:], baa_t)
                # LayerNorm over features
                mu = cpool2.tile([P, 1], f32, tag="mu")
                nc.vector.tensor_reduce(mu[:], o1[:], axis=mybir.AxisListType.X,
                                        op=OP.add, negate=True)
                nc.vector.tensor_scalar_mul(mu[:], mu[:], 1.0 / D)
                xm = cpool2.tile([P, D], f32, tag="xm")
                nc.vector.tensor_scalar_add(xm[:], o1[:], mu[:, 0:1])
                sq = cpool2.tile([P, D], f32, tag="sq")
                var = cpool2.tile([P, 1], f32, tag="var")
                nc.scalar.activation(sq[:], xm[:], AF.Square,
                                     accum_out=var[:, 0:1])
                std = cpool2.tile([P, 1], f32, tag="std")
                nc.scalar.activation(std[:], var[:], AF.Sqrt, scale=1.0 / D,
                                     bias=LN_EPS)
                rinv = cpool2.tile([P, 1], f32, tag="rinv")
                nc.vector.reciprocal(rinv[:], std[:])
                xn = cpool2.tile([P, D], f32, tag="xn")
                nc.vector.tensor_scalar_mul(xn[:], xm[:], rinv[:, 0:1])
                ot = cpool2.tile([P, D], f32, tag="ot")
                nc.vector.tensor_mul(ot[:], xn[:], gam_t)
                nc.vector.tensor_add(ot[:], ot[:], bet_t)
                # per-row int8 quantization: q = round(ot * 127/amax)
                oab = cpool2.tile([P, D], f32, tag="oab")
                nc.scalar.activation(oab[:], ot[:], AF.Abs)
                oam = cpool2.tile([P, 1], f32, tag="oam")
                nc.vector.tensor_reduce(oam[:], oab[:], axis=mybir.AxisListType.X,
                                        op=OP.max)
                osc = cpool2.tile([P, 1], f32, tag="osc")
                nc.vector.tensor_scalar_max(osc[:], oam[:], 1e-30)
                nc.vector.tensor_scalar_mul(osc[:], osc[:], 1.0 / 127.0)
                oinv = cpool2.tile([P, 1], f32, tag="oinv")
                nc.vector.reciprocal(oinv[:], osc[:])
                oq = cpool2.tile([P, D], i8, tag="oq")
                nc.scalar.activation(oq[:], ot[:], AF.Copy, scale=oinv[:, 0:1])
                osh = cpool2.tile([P, 1], f16, tag="osh")
                nc.scalar.copy(osh[:], osc[:])
                nc.sync.dma_start(out_q[w * P:w * P + nt, :], oq[:nt])
                nc.sync.dma_start(out_s[w * P:w * P + nt, :], osh[:nt])
            stkC.close()

    nc.compile()
    return nc


def _make_runner(nc):
    """Build the cached PJRT callable (the same lowering run_bass_kernel_spmd
    uses under axon, minus the per-call retrace and the zeros upload)."""
    import jax
    from jax.sharding import Mesh, PartitionSpec, NamedSharding
    from jax.experimental.shard_map import shard_map
    from concourse import bass2jax, mybir

    bass2jax.install_neuronx_cc_hook()
    partition_name = nc.partition_id_tensor.name if nc.partition_id_tensor else None
    in_names, out_names, out_avals = [], [], []
    for alloc in nc.m.functions[0].allocations:
        if not isinstance(alloc, mybir.MemoryLocationSet):
            continue
        name = alloc.memorylocations[0].name
        if alloc.kind == "ExternalInput":
            if name != partition_name:
                in_names.append(name)
        elif alloc.kind == "ExternalOutput":
            out_names.append(name)
            out_avals.append(jax.core.ShapedArray(
                tuple(alloc.tensor_shape), mybir.dt.np(alloc.dtype)))
    n_params = len(in_names)
    n_outs = len(out_names)
    in_names_all = in_names + out_names
    if partition_name is not None:
        in_names_all.append(partition_name)

    def _body(*args):
        operands = list(args)
        if partition_name is not None:
            operands.append(bass2jax.partition_id_tensor())
        outs = bass2jax._bass_exec_p.bind(
            *operands,
            out_avals=tuple(out_avals),
            in_names=tuple(in_names_all),
            out_names=tuple(out_names),
            lowering_input_output_aliases=(),
            sim_require_finite=True,
            sim_require_nnan=True,
            nc=nc,
        )
        return tuple(outs)

    devices = jax.devices()[:NCORES]
    mesh = Mesh(np.asarray(devices), ("core",))
    sharded = jax.jit(
        shard_map(_body, mesh=mesh,
                  in_specs=(PartitionSpec("core"),) * (n_params + n_outs),
                  out_specs=(PartitionSpec("core"),) * n_outs,
                  check_rep=False),
        keep_unused=True)
    # Output operands only feed the parameter-order check; the kernel writes
    # every element of `out`, so their content is irrelevant -> keep one
    # device-resident buffer alive and pass it every call (no upload).
    sh = NamedSharding(mesh, PartitionSpec("core"))
    dummy = [jax.device_put(
        np.zeros((NCORES * a.shape[0], *a.shape[1:]), a.dtype), sh)
        for a in out_avals]

    def run(in_maps):
        concat_in = [
            np.concatenate([np.asarray(m[name]) for m in in_maps], axis=0)
            for name in in_names]
        outs = sharded(*concat_in, *dummy)
        fetched = jax.device_get(list(outs))
        return dict(zip(out_names, fetched))

    return run


_CACHE = {}


def _get_compiled(meta):
    key = (meta["n"], meta["npc"], meta["nwin"], meta["bpw"], meta["alpha"])
    if key not in _CACHE:
        nc = _build(meta)
        _CACHE[key] = _make_runner(nc)
    return _CACHE[key]


def kernel(**inputs):
    in_maps, meta = _host_prep(**inputs)
    run = _get_compiled(meta)
    outs = run(in_maps)
    return outs["out_q"].astype(np.float32) * outs["out_s"].astype(np.float32)


# revision 15
# speedup vs baseline: 2.0436x; 1.0527x over previous
"""HGT graph update kernel for 8 Trainium2 NeuronCores.

Strategy:
  * Host folds the per-relation projections into node-level weights:
      kt_s = x @ (Wk @ blockdiag(Watt_s)) * prior_s/sqrt(C)   (per head col-block)
      mt_s = x @ (Wm @ blockdiag(Wmsg_s))
    so each edge only needs gathers:  score = <kt_s[src], q[dst]>_per-head,
    msg = mt_s[src].
  * Softmax without the max-subtraction pass (scores are O(1) here; the
    shifted/unshifted softmax are algebraically identical, fp32-safe).
  * All 2E edges are sorted by destination on the host; the 8 cores own
    contiguous 12500-node ranges, so each core completes its own segment
    softmax locally - the only collective is one AllGather of the node
    tables kt/mt (q stays core-local in SBUF).
  * Edge phase: per 128-edge block, one indirect DMA gathers [kt|mt]
    (1024B/edge) from the gathered table; q[dst] is reconstructed with a
    one-hot matmul from SBUF (no DMA); scatter-add into a PSUM window of
    128 consecutive dst nodes via a one-hot matmul.

  Host<->device transport (the axon tunnel, ~50MB/s) dominates, so:
  * x and the output travel as per-row int8 (amax/127 fp16 row scales).
    Tolerance is 2e-2; each int8 leg adds ~6e-3 (HW f32->int8 conversion
    is exact round-to-nearest-even, verified on device).
  * Each edge is packed into ONE int32: kmidx (18b) | rowlocal (8b).
    rowlocal is unpacked on device (shift + int->f32 copy); the one-hot
    selector transpose is done on TensorE instead of shipping a second
    pre-transposed copy of the indices.
  * Weights/biases ship as fp16, biases as single rows broadcast on
    device via a ones-outer-product matmul.
  * The jitted PJRT callable is built once and cached; the output-operand
    required by the bass_exec custom call is a persistent device-resident
    dummy (the kernel writes every output element), so no per-call zeros
    upload.
"""

import sys

if "/opt/trn_rl_repo" not in sys.path:
    sys.path.insert(0, "/opt/trn_rl_repo")
import numpy as np

N, D, H, C = 100000, 128, 8, 16
LN_EPS = 1e-3
NCORES = 8
P = 128
ROWSHIFT = 18                 # kmidx bits 0..17, rowlocal bits 18..25
KMASK = (1 << ROWSHIFT) - 1
DUMMY_ROW = 255               # never matches a 0..127 local row


def _host_prep(x, src0, dst0, src1, dst1, Wk, bk, Wm, bm, Wq, bq, Wa, ba,
               Watt0, Wmsg0, Watt1, Wmsg1, prior0, prior1, skip, gamma, beta):
    """Fold weights, sort edges by dst, build per-core packed index records."""
    f32 = np.float32
    x = np.asarray(x, f32)
    n = x.shape[0]
    npc = n // NCORES            # nodes per core
    nwin = (npc + P - 1) // P    # windows (128-node groups) per core

    def bd(w):  # [H,C,C] -> block-diagonal [D,D]
        out = np.zeros((H * C, H * C), f32)
        for h in range(H):
            out[h * C:(h + 1) * C, h * C:(h + 1) * C] = np.asarray(w[h], f32)
        return out

    scale = 1.0 / np.sqrt(f32(C))
    cs0 = np.repeat(np.asarray(prior0, f32) * scale, C)   # [D] col scale
    cs1 = np.repeat(np.asarray(prior1, f32) * scale, C)
    Wk, bk, Wm, bm = (np.asarray(a, f32) for a in (Wk, bk, Wm, bm))
    Wkt0 = (Wk @ bd(Watt0)) * cs0; bkt0 = (bk @ bd(Watt0)) * cs0
    Wkt1 = (Wk @ bd(Watt1)) * cs1; bkt1 = (bk @ bd(Watt1)) * cs1
    Wmt0 = Wm @ bd(Wmsg0); bmt0 = bm @ bd(Wmsg0)
    Wmt1 = Wm @ bd(Wmsg1); bmt1 = bm @ bd(Wmsg1)
    # T row layout per node: [kt0 | mt0 | kt1 | mt1]  -> viewed as [2n, 256]:
    # row 2s+b = [kt_b | mt_b] of node s.
    Wbig = np.concatenate([Wkt0, Wmt0, Wkt1, Wmt1], axis=1)        # [128, 512]
    bbig = np.concatenate([bkt0, bmt0, bkt1, bmt1])                # [512]

    # ---- edges: sort by dst ----
    e0 = len(np.asarray(src0)); e1 = len(np.asarray(src1))
    src = np.concatenate([np.asarray(src0), np.asarray(src1)]).astype(np.int64)
    dst = np.concatenate([np.asarray(dst0), np.asarray(dst1)]).astype(np.int64)
    eset = np.concatenate([np.zeros(e0, np.int64), np.ones(e1, np.int64)])
    order = np.argsort(dst, kind="stable")
    src, dst, eset = src[order], dst[order], eset[order]
    kmidx = (2 * src + eset).astype(np.int64)      # row into [2n, 256] table

    # per-core, per-window edge ranges: window w of core c covers nodes
    # [c*npc + w*P, next boundary); boundaries are strictly increasing so a
    # single searchsorted over all of them yields every window's edge range.
    win_start = np.asarray(
        [c * npc + w * P for c in range(NCORES) for w in range(nwin)], np.int64)
    starts = np.searchsorted(dst, np.concatenate([win_start, [n]]), side="left")
    cnts = np.diff(starts)                           # edges per window
    bpw = max(1, int((cnts.max() + P - 1) // P))

    # packed record: word = kmidx | rowlocal<<18, block-major within window
    rowloc = dst - np.repeat(win_start, cnts)
    word_all = (kmidx | (rowloc << ROWSHIFT)).astype(np.int32)
    wrec = np.full((NCORES * nwin, bpw * P), DUMMY_ROW << ROWSHIFT, np.int32)
    mask = np.arange(bpw * P)[None, :] < cnts[:, None]
    wrec[mask] = word_all
    # [nwin*P? ] -> per window [bpw, P] -> transpose to [P, bpw]
    wrec = wrec.reshape(NCORES, nwin, bpw, P).transpose(0, 1, 3, 2)
    wrec = np.ascontiguousarray(wrec)

    alpha = float(1.0 / (1.0 + np.exp(-np.float64(np.asarray(skip)))))
    f16 = np.float16
    wall = np.concatenate(
        [Wbig, np.asarray(Wq, f32), np.asarray(Wa, f32)], axis=1).astype(f16)
    crow = np.concatenate(
        [bbig, np.asarray(bq, f32), np.asarray(ba, f32) * f32(alpha),
         np.asarray(gamma, f32), np.asarray(beta, f32)])[None, :].astype(f16)
    consts = dict(wall=wall, crow=crow)
    # per-row int8 quantization of x (device dequantizes with the fp16 scale)
    xs = (np.maximum(np.abs(x).max(axis=1, keepdims=True), 1e-30)
          / f32(127.0)).astype(f16)
    xq = np.clip(np.round(x / xs.astype(f32)), -127, 127).astype(np.int8)
    in_maps = []
    for c in range(NCORES):
        m = dict(consts)
        m["x_q"] = np.ascontiguousarray(xq[c * npc:(c + 1) * npc])
        m["x_s"] = np.ascontiguousarray(xs[c * npc:(c + 1) * npc])
        m["wrec"] = wrec[c]
        in_maps.append(m)
    return in_maps, dict(n=n, npc=npc, nwin=nwin, bpw=bpw, alpha=alpha)


def _build(meta):
    """Build the Bass program (shared by all 8 cores)."""
    import concourse.bass as bass
    import concourse.mybir as mybir
    import concourse.tile as tile
    from concourse.masks import make_identity

    f32 = mybir.dt.float32
    f16 = mybir.dt.float16
    i32 = mybir.dt.int32
    AF = mybir.ActivationFunctionType
    OP = mybir.AluOpType
    n, npc, nwin, bpw = meta["n"], meta["npc"], meta["nwin"], meta["bpw"]
    alpha = meta["alpha"]

    import concourse.bacc as bacc
    nc = bacc.Bacc(trn_type="TRN2", num_devices=NCORES)

    i8 = mybir.dt.int8
    x_q = nc.dram_tensor("x_q", [npc, D], i8, kind="ExternalInput")
    x_s = nc.dram_tensor("x_s", [npc, 1], f16, kind="ExternalInput")
    wrec = nc.dram_tensor("wrec", [nwin, P, bpw], i32, kind="ExternalInput")
    wall = nc.dram_tensor("wall", [D, 6 * D], f16, kind="ExternalInput")
    crow = nc.dram_tensor("crow", [1, 8 * D], f16, kind="ExternalInput")
    out_q = nc.dram_tensor("out_q", [npc, D], i8, kind="ExternalOutput")
    out_s = nc.dram_tensor("out_s", [npc, 1], f16, kind="ExternalOutput")

    from contextlib import ExitStack
    with tile.TileContext(nc, num_cores=NCORES) as tc:
        with (
            tc.tile_pool(name="const", bufs=1) as cpool,
            tc.tile_pool(name="dram", bufs=1, space="DRAM") as dram,
        ):
            # ---- constants ----
            identity = cpool.tile([P, P], f32)
            make_identity(nc, identity[:])
            iota_free = cpool.tile([P, P], f32)
            nc.gpsimd.iota(iota_free[:], pattern=[[1, P]], channel_multiplier=0,
                           allow_small_or_imprecise_dtypes=True)
            ones_row = cpool.tile([1, P], f32)
            nc.vector.memset(ones_row[:], 1.0)
            zero_col = cpool.tile([P, 1], f32)
            nc.vector.memset(zero_col[:], 0.0)
            eps_col = cpool.tile([P, 1], f32)
            nc.vector.memset(eps_col[:], LN_EPS)
            nc.const_aps.aps[(f32, 0.0)] = zero_col[:]
            nc.const_aps.aps[(f32, LN_EPS)] = eps_col[:]

            wall_h = cpool.tile([D, 6 * D], f16)
            nc.sync.dma_start(wall_h[:], wall[:])
            wall_t = cpool.tile([D, 6 * D], f32)
            nc.scalar.copy(wall_t[:], wall_h[:])
            wbig_t = wall_t[:][:, 0:4 * D]
            wq_t = wall_t[:][:, 4 * D:5 * D]
            wa_t = wall_t[:][:, 5 * D:6 * D]

            crow_h = cpool.tile([1, 8 * D], f16)
            nc.sync.dma_start(crow_h[:], crow[:])
            crow_f = cpool.tile([1, 8 * D], f32)
            nc.scalar.copy(crow_f[:], crow_h[:])
            cbrd = cpool.tile([P, 8 * D], f32)
            with tc.tile_pool(name="bc_ps", bufs=2, space="PSUM") as bcps:
                for j in range(2):
                    ps = bcps.tile([P, 4 * D], f32, tag="bc")
                    nc.tensor.matmul(ps[:], lhsT=ones_row[:],
                                     rhs=crow_f[:][:, j * 4 * D:(j + 1) * 4 * D],
                                     start=True, stop=True)
                    nc.scalar.copy(cbrd[:, j * 4 * D:(j + 1) * 4 * D], ps[:])
            bb_t = cbrd[:][:, 0:4 * D]
            bq_t = cbrd[:][:, 4 * D:5 * D]
            baa_t = cbrd[:][:, 5 * D:6 * D]
            gam_t = cbrd[:][:, 6 * D:7 * D]
            bet_t = cbrd[:][:, 7 * D:8 * D]

            # persistent SBUF state
            q_sbuf = cpool.tile([P, nwin * D], f32)
            nc.gpsimd.memset(q_sbuf[:], 0)
            pooled = cpool.tile([P, nwin * 136], f32)

            T_local = dram.tile([npc, 4 * D], f32)
            T_full = dram.tile([2 * n, 2 * D], f32)

            # ================= Phase A: projections =================
            stkA = ExitStack()
            apool = stkA.enter_context(tc.tile_pool(name="a_sb", bufs=3))
            apsum = stkA.enter_context(tc.tile_pool(name="a_ps", bufs=2, space="PSUM"))
            for t in range(nwin):
                nt = min(P, npc - t * P)
                xq8 = apool.tile([P, D], i8, tag="xq8")
                xsh = apool.tile([P, 1], f16, tag="xsh")
                if nt < P:
                    nc.vector.memset(xq8[:], 0)
                    nc.vector.memset(xsh[:], 0)
                nc.sync.dma_start(xq8[:nt], x_q[t * P:t * P + nt, :])
                nc.sync.dma_start(xsh[:nt], x_s[t * P:t * P + nt, :])
                xsf = apool.tile([P, 1], f32, tag="xsf")
                nc.scalar.copy(xsf[:], xsh[:])
                xt = apool.tile([P, D], f32, tag="xt")
                nc.scalar.activation(xt[:], xq8[:], AF.Copy, scale=xsf[:, 0:1])
                xT_ps = apsum.tile([P, P], f32, tag="xT")
                nc.tensor.transpose(xT_ps[:], xt[:], identity[:])
                xTs = apool.tile([P, P], f32, tag="xTs")
                nc.scalar.copy(xTs[:], xT_ps[:])
                T_ps = apsum.tile([P, 4 * D], f32, tag="Tps")
                nc.tensor.matmul(T_ps[:], lhsT=xTs[:], rhs=wbig_t,
                                 start=True, stop=True)
                Tb = apool.tile([P, 4 * D], f32, tag="Tb")
                nc.vector.tensor_add(Tb[:], T_ps[:], bb_t)
                nc.sync.dma_start(T_local[t * P:t * P + nt, :], Tb[:nt])
                q_ps = apsum.tile([P, D], f32, tag="qps")
                nc.tensor.matmul(q_ps[:], lhsT=xTs[:], rhs=wq_t,
                                 start=True, stop=True)
                nc.vector.tensor_add(q_sbuf[:nt, t * D:(t + 1) * D],
                                     q_ps[:nt], bq_t[:nt])

            stkA.close()

            # ================= AllGather node tables =================
            nc.gpsimd.collective_compute(
                "AllGather",
                mybir.AluOpType.bypass,
                replica_groups=[list(range(NCORES))],
                ins=[T_local[:]],
                outs=[T_full[:]],
            )

            # ================= Phase B: edges =================
            stkB = ExitStack()
            bpool = stkB.enter_context(tc.tile_pool(name="b_sb", bufs=4))
            bpsum = stkB.enter_context(tc.tile_pool(name="b_ps", bufs=3, space="PSUM"))
            wpsum = stkB.enter_context(tc.tile_pool(name="win_ps", bufs=2, space="PSUM"))
            for w in range(nwin):
                wr = bpool.tile([P, bpw], i32, tag="wr")
                nc.sync.dma_start(wr[:], wrec[w, :, :])
                km = bpool.tile([P, bpw], i32, tag="km")
                nc.vector.tensor_scalar(km[:], wr[:], KMASK, None,
                                        op0=OP.bitwise_and)
                rli = bpool.tile([P, bpw], i32, tag="rli")
                nc.vector.tensor_scalar(rli[:], wr[:], ROWSHIFT, None,
                                        op0=OP.logical_shift_right)
                rlf = bpool.tile([P, bpw], f32, tag="rlf")
                nc.scalar.copy(rlf[:], rli[:])
                win_ps = wpsum.tile([P, 136], f32, tag="win")
                for b in range(bpw):
                    ktmt = bpool.tile([P, 2 * D], f32, tag="ktmt", bufs=8)
                    nc.gpsimd.indirect_dma_start(
                        out=ktmt[:], out_offset=None,
                        in_=T_full[:],
                        in_offset=bass.IndirectOffsetOnAxis(
                            ap=km[:, b:b + 1], axis=0),
                    )
                    # Sel[e,j] = (rowlocal_e == j)
                    sel = bpool.tile([P, P], f32, tag="sel")
                    nc.vector.tensor_scalar(
                        sel[:], iota_free[:], rlf[:, b:b + 1], None,
                        op0=OP.is_equal)
                    # SelT[j,e] = (j == rowlocal_e)  (TensorE transpose)
                    selT_ps = bpsum.tile([P, P], f32, tag="selTp")
                    nc.tensor.transpose(selT_ps[:], sel[:], identity[:])
                    selT = bpool.tile([P, P], f32, tag="selT")
                    nc.scalar.copy(selT[:], selT_ps[:])
                    # q[dst] for each edge
                    qe_ps = bpsum.tile([P, P], f32, tag="qe")
                    nc.tensor.matmul(qe_ps[:], lhsT=selT[:],
                                     rhs=q_sbuf[:, w * D:(w + 1) * D],
                                     start=True, stop=True)
                    prod = bpool.tile([P, D], f32, tag="prod")
                    nc.vector.tensor_mul(prod[:], ktmt[:][:, 0:D], qe_ps[:])
                    rhs = bpool.tile([P, 136], f32, tag="rhs")
                    nc.vector.tensor_reduce(
                        rhs[:, D:D + H], prod[:].rearrange("p (h c) -> p h c", c=C),
                        axis=mybir.AxisListType.X, op=OP.add)
                    nc.scalar.activation(rhs[:, D:D + H], rhs[:, D:D + H], AF.Exp)
                    nc.vector.tensor_tensor(
                        rhs[:, 0:D].rearrange("p (h c) -> p h c", c=C),
                        ktmt[:][:, D:2 * D].rearrange("p (h c) -> p h c", c=C),
                        rhs[:, D:D + H].rearrange("p (h o) -> p h o", o=1)
                            .to_broadcast([P, H, C]),
                        op=OP.mult)
                    nc.tensor.matmul(win_ps[:], lhsT=sel[:], rhs=rhs[:],
                                     start=(b == 0), stop=(b == bpw - 1))
                nc.scalar.copy(pooled[:, w * 136:(w + 1) * 136], win_ps[:])

            stkB.close()

            # ================= Phase C: aggregate + LN =================
            stkC = ExitStack()
            cpool2 = stkC.enter_context(tc.tile_pool(name="c_sb", bufs=3))
            cpsum = stkC.enter_context(tc.tile_pool(name="c_ps", bufs=2, space="PSUM"))
            for w in range(nwin):
                nt = min(P, npc - w * P)
                num = pooled[:, w * 136:w * 136 + D]
                den = pooled[:, w * 136 + D:w * 136 + D + H]
                denc = cpool2.tile([P, H], f32, tag="denc")
                nc.vector.tensor_scalar_max(denc[:], den, 1e-30)
                inv = cpool2.tile([P, H], f32, tag="inv")
                nc.vector.reciprocal(inv[:], denc[:])
                pn = cpool2.tile([P, D], f32, tag="pn")
                nc.vector.tensor_tensor(
                    pn[:].rearrange("p (h c) -> p h c", c=C),
                    num.rearrange("p (h c) -> p h c", c=C),
                    inv[:].rearrange("p (h o) -> p h o", o=1)
                        .to_broadcast([P, H, C]),
                    op=OP.mult)
                g = cpool2.tile([P, D], f32, tag="g")
                nc.scalar.activation(g[:], pn[:], AF.Gelu)
                gT_ps = cpsum.tile([P, P], f32, tag="gT")
                nc.tensor.transpose(gT_ps[:], g[:], identity[:])
                gTs = cpool2.tile([P, P], f32, tag="gTs")
                nc.scalar.copy(gTs[:], gT_ps[:])
                h_ps = cpsum.tile([P, D], f32, tag="hps")
                nc.tensor.matmul(h_ps[:], lhsT=gTs[:], rhs=wa_t,
                                 start=True, stop=True)
                xq2 = cpool2.tile([P, D], i8, tag="xq2")
                nc.sync.dma_start(xq2[:nt], x_q[w * P:w * P + nt, :])
                xsh2 = cpool2.tile([P, 1], f16, tag="xsh2")
                nc.sync.dma_start(xsh2[:nt], x_s[w * P:w * P + nt, :])
                xsa = cpool2.tile([P, 1], f32, tag="xsa")
                nc.scalar.activation(xsa[:], xsh2[:], AF.Copy, scale=1.0 - alpha)
                o1 = cpool2.tile([P, D], f32, tag="o1")
                nc.vector.tensor_scalar_mul(o1[:], h_ps[:], alpha)
                xt2 = cpool2.tile([P, D], f32, tag="xt2")
                nc.scalar.activation(xt2[:], xq2[:], AF.Copy, scale=xsa[:, 0:1])
                nc.vector.tensor_add(o1[:], o1[:], xt2[:])
                nc.vector.tensor_add(o1[:], o1[:], baa_t)
                # LayerNorm over features
                mu = cpool2.tile([P, 1], f32, tag="mu")
                nc.vector.tensor_reduce(mu[:], o1[:], axis=mybir.AxisListType.X,
                                        op=OP.add, negate=True)
                nc.vector.tensor_scalar_mul(mu[:], mu[:], 1.0 / D)
                xm = cpool2.tile([P, D], f32, tag="xm")
                nc.vector.tensor_scalar_add(xm[:], o1[:], mu[:, 0:1])
                sq = cpool2.tile([P, D], f32, tag="sq")
                var = cpool2.tile([P, 1], f32, tag="var")
                nc.scalar.activation(sq[:], xm[:], AF.Square,
                                     accum_out=var[:, 0:1])
                std = cpool2.tile([P, 1], f32, tag="std")
                nc.scalar.activation(std[:], var[:], AF.Sqrt, scale=1.0 / D,
                                     bias=LN_EPS)
                rinv = cpool2.tile([P, 1], f32, tag="rinv")
                nc.vector.reciprocal(rinv[:], std[:])
                xn = cpool2.tile([P, D], f32, tag="xn")
                nc.vector.tensor_scalar_mul(xn[:], xm[:], rinv[:, 0:1])
                ot = cpool2.tile([P, D], f32, tag="ot")
                nc.vector.tensor_mul(ot[:], xn[:], gam_t)
                nc.vector.tensor_add(ot[:], ot[:], bet_t)
                # per-row int8 quantization: q = round(ot * 127/amax)
                oab = cpool2.tile([P, D], f32, tag="oab")
                nc.scalar.activation(oab[:], ot[:], AF.Abs)
                oam = cpool2.tile([P, 1], f32, tag="oam")
                nc.vector.tensor_reduce(oam[:], oab[:], axis=mybir.AxisListType.X,
                                        op=OP.max)
                osc = cpool2.tile([P, 1], f32, tag="osc")
                nc.vector.tensor_scalar_max(osc[:], oam[:], 1e-30)
                nc.vector.tensor_scalar_mul(osc[:], osc[:], 1.0 / 127.0)
                oinv = cpool2.tile([P, 1], f32, tag="oinv")
                nc.vector.reciprocal(oinv[:], osc[:])
                oq = cpool2.tile([P, D], i8, tag="oq")
                nc.scalar.activation(oq[:], ot[:], AF.Copy, scale=oinv[:, 0:1])
                osh = cpool2.tile([P, 1], f16, tag="osh")
                nc.scalar.copy(osh[:], osc[:])
                nc.sync.dma_start(out_q[w * P:w * P + nt, :], oq[:nt])
                nc.sync.dma_start(out_s[w * P:w * P + nt, :], osh[:nt])
            stkC.close()

    nc.compile()
    return nc


def _make_runner(nc):
    """Build the cached PJRT callable (the same lowering run_bass_kernel_spmd
    uses under axon, minus the per-call retrace and the zeros upload)."""
    import jax
    from jax.sharding import Mesh, PartitionSpec, NamedSharding
    from jax.experimental.shard_map import shard_map
    from concourse import bass2jax, mybir

    bass2jax.install_neuronx_cc_hook()
    partition_name = nc.partition_id_tensor.name if nc.partition_id_tensor else None
    in_names, out_names, out_avals = [], [], []
    for alloc in nc.m.functions[0].allocations:
        if not isinstance(alloc, mybir.MemoryLocationSet):
            continue
        name = alloc.memorylocations[0].name
        if alloc.kind == "ExternalInput":
            if name != partition_name:
                in_names.append(name)
        elif alloc.kind == "ExternalOutput":
            out_names.append(name)
            out_avals.append(jax.core.ShapedArray(
                tuple(alloc.tensor_shape), mybir.dt.np(alloc.dtype)))
    n_params = len(in_names)
    n_outs = len(out_names)
    in_names_all = in_names + out_names
    if partition_name is not None:
        in_names_all.append(partition_name)

    def _body(*args):
        operands = list(args)
        if partition_name is not None:
            operands.append(bass2jax.partition_id_tensor())
        outs = bass2jax._bass_exec_p.bind(
            *operands,
            out_avals=tuple(out_avals),
            in_names=tuple(in_names_all),
            out_names=tuple(out_names),
            lowering_input_output_aliases=(),
            sim_require_finite=True,
            sim_require_nnan=True,
            nc=nc,
        )
        return tuple(outs)

    devices = jax.devices()[:NCORES]
    mesh = Mesh(np.asarray(devices), ("core",))
    # wall/crow are identical on every core: pass them replicated (one host
    # copy) instead of concatenating 8 copies into the sharded upload.
    replicated = {"wall", "crow"}
    in_specs = tuple(
        PartitionSpec() if name in replicated else PartitionSpec("core")
        for name in in_names) + (PartitionSpec("core"),) * n_outs
    sharded = jax.jit(
        shard_map(_body, mesh=mesh,
                  in_specs=in_specs,
                  out_specs=(PartitionSpec("core"),) * n_outs,
                  check_rep=False),
        keep_unused=True)
    # Output operands only feed the parameter-order check; the kernel writes
    # every element of `out`, so their content is irrelevant -> keep one
    # device-resident buffer alive and pass it every call (no upload).
    sh = NamedSharding(mesh, PartitionSpec("core"))
    dummy = [jax.device_put(
        np.zeros((NCORES * a.shape[0], *a.shape[1:]), a.dtype), sh)
        for a in out_avals]

    def run(in_maps):
        concat_in = [
            np.asarray(in_maps[0][name]) if name in replicated
            else np.concatenate([np.asarray(m[name]) for m in in_maps], axis=0)
            for name in in_names]
        outs = sharded(*concat_in, *dummy)
        fetched = jax.device_get(list(outs))
        return dict(zip(out_names, fetched))

    return run


_CACHE = {}


def _get_compiled(meta):
    key = (meta["n"], meta["npc"], meta["nwin"], meta["bpw"], meta["alpha"])
    if key not in _CACHE:
        nc = _build(meta)
        _CACHE[key] = _make_runner(nc)
    return _CACHE[key]


def kernel(**inputs):
    in_maps, meta = _host_prep(**inputs)
    run = _get_compiled(meta)
    outs = run(in_maps)
    return outs["out_q"].astype(np.float32) * outs["out_s"].astype(np.float32)
